# revision 1
# baseline (speedup 1.0000x reference)
"""Trainium2 Bass kernel for nn_MultiHeadAttention_75737453297867.

Sharding: one head per NeuronCore (8 heads / 8 cores). The reference's
aliased as_strided gather needs a per-core base offset 24576*h into the
flange-padded k/v storage; 24576*h mod 9216 is always row-aligned
(phi in {0,48,96} rows of the padded [144,64] channel), so each core
statically builds three phi-shifted staging variants and the active one
is selected purely through host data: the QK contraction runs over a
stacked K=12 (q-conv weights zeroed for inactive variants) and uv goes
through a one-hot select matmul. No runtime-dependent addressing.

Per core: q conv (12 stacked ch), k/v convs (4 ch slice c_lo..c_lo+4,
k pre-scaled by DPH^-0.5), 3-variant staging in DRAM, static gathers,
block attention (QK on PE fp32r, exp on ACT from PSUM, AV on PE with a
ones-row denominator), final conv with wo[:, 4h:4h+4]. Host sums the 8
partial outputs.
"""

import sys

import numpy as np

if "/opt/trn_rl_repo" not in sys.path:
    sys.path.insert(0, "/opt/trn_rl_repo")

import concourse.bass as bass
import concourse.tile as tile
from concourse import bacc
from concourse import mybir
from concourse.bass_types import AP

# Problem constants
CIN, COUT, H, W = 64, 64, 128, 48
DM, NH, DPH = 32, 8, 4
Q0, Q1, F0, F1 = 128, 24, 8, 8
M0, M1 = Q0 + 2 * F0, Q1 + 2 * F1          # 144, 40
HP, WP = H + 2 * F0, W + 2 * F1            # 144, 64
KV = M0 * M1                               # 5760
NKC = 48                                   # kv chunks of 120 (3 m0-rows)
KC = 120
NQC = 8                                    # q chunks of 384 (16 rows x 24 cols)
QC = 384
QROWS = 16
RPB = 3                                    # kv chunks per psum round (3 banks, double-buffered)
PHIS = (0, 48, 96)
F32 = mybir.dt.float32
F32R = mybir.dt.float32r
I32 = mybir.dt.int32


def build_nc(debug=False):
    nc = bacc.Bacc()
    dbg = {}
    if debug:
        dbg['q'] = nc.dram_tensor("dbg_q", [12, H * W], F32, kind="ExternalOutput")
        dbg['k'] = nc.dram_tensor("dbg_k", [4, H * W], F32, kind="ExternalOutput")
        dbg['uk0'] = nc.dram_tensor("dbg_uk0", [12, KV], F32, kind="ExternalOutput")
        dbg['uvt0'] = nc.dram_tensor("dbg_uvt0", [KC, NKC * 5], F32, kind="ExternalOutput")
        dbg['opad'] = nc.dram_tensor("dbg_opad", [4, 130 * 50], F32, kind="ExternalOutput")

    xp_d = nc.dram_tensor("xp", [CIN, 130 * 50], F32R, kind="ExternalInput")
    wq_d = nc.dram_tensor("wq_t", [CIN, 9 * 12], F32R, kind="ExternalInput")
    wk_d = nc.dram_tensor("wk_t", [CIN, 9 * 4], F32R, kind="ExternalInput")
    wv_d = nc.dram_tensor("wv_t", [CIN, 9 * 4], F32R, kind="ExternalInput")
    wo_d = nc.dram_tensor("wo_t", [5, 9 * 64], F32R, kind="ExternalInput")
    bq_d = nc.dram_tensor("bq_l", [12, 1], F32, kind="ExternalInput")
    bk_d = nc.dram_tensor("bk_l", [4, 1], F32, kind="ExternalInput")
    bv_d = nc.dram_tensor("bv_l", [4, 1], F32, kind="ExternalInput")
    sel_d = nc.dram_tensor("sel", [12, 4], F32R, kind="ExternalInput")
    id_d = nc.dram_tensor("ident4", [4, 4], F32, kind="ExternalInput")
    cc_d = nc.dram_tensor("concol", [128, 2], F32, kind="ExternalInput")
    o14_d = nc.dram_tensor("ones14", [1, 5], F32R, kind="ExternalInput")
    out_d = nc.dram_tensor("out", [COUT, H * W], F32, kind="ExternalOutput")

    from contextlib import ExitStack

    with tile.TileContext(nc) as tc, ExitStack() as ctx:
        P = ctx.enter_context(tc.tile_pool(name="persist", bufs=1))
        dram = ctx.enter_context(tc.tile_pool(name="dram", bufs=1, space="DRAM"))
        ctx1 = ctx.enter_context(ExitStack())
        P1 = ctx1.enter_context(tc.tile_pool(name="phase1", bufs=1))

        # ---- load constants ----
        xp_sb = P1.tile([CIN, 130, 50], F32R, tag="xp")
        nc.sync.dma_start(out=xp_sb, in_=xp_d[:, :].rearrange("p (a b) -> p a b", a=130))
        wq_sb = P.tile([CIN, 9, 12], F32R, tag="wq")
        nc.sync.dma_start(out=wq_sb, in_=wq_d[:, :].rearrange("p (t o) -> p t o", t=9))
        wk_sb = P.tile([CIN, 9, 4], F32R, tag="wk")
        nc.sync.dma_start(out=wk_sb, in_=wk_d[:, :].rearrange("p (t o) -> p t o", t=9))
        wv_sb = P.tile([CIN, 9, 4], F32R, tag="wv")
        nc.sync.dma_start(out=wv_sb, in_=wv_d[:, :].rearrange("p (t o) -> p t o", t=9))
        wo_sb = P.tile([5, 9, 64], F32R, tag="wo")
        nc.sync.dma_start(out=wo_sb, in_=wo_d[:, :].rearrange("p (t o) -> p t o", t=9))
        bq_sb = P.tile([12, 1], F32, tag="bq")
        nc.sync.dma_start(out=bq_sb, in_=bq_d[:, :])
        bk_sb = P.tile([4, 1], F32, tag="bk")
        nc.sync.dma_start(out=bk_sb, in_=bk_d[:, :])
        bv_sb = P.tile([4, 1], F32, tag="bv")
        nc.sync.dma_start(out=bv_sb, in_=bv_d[:, :])
        sel_sb = P.tile([12, 4], F32R, tag="sel")
        nc.sync.dma_start(out=sel_sb, in_=sel_d[:, :])
        ident = P.tile([4, 4], F32, tag="ident")
        nc.sync.dma_start(out=ident, in_=id_d[:, :])
        concol = P.tile([128, 2], F32, tag="concol")
        nc.sync.dma_start(out=concol, in_=cc_d[:, :])
        ones14 = P.tile([1, 5], F32R, tag="ones14")
        nc.sync.dma_start(out=ones14, in_=o14_d[:, :])

        # ---- convs: q (12ch stacked), k (4ch, pre-scaled), v (4ch) ----
        q_sb = P.tile([12, 128, 48], F32R, tag="q_sb")
        k_sb = P1.tile([4, 128, 48], F32, tag="k_sb")
        v_sb = P1.tile([4, 128, 48], F32, tag="v_sb")

        with tc.tile_pool(name="psc", bufs=4, space="PSUM") as psc:
            for w_sb, b_sb, m, dst in (
                (wq_sb, bq_sb, 12, q_sb),
                (wk_sb, bk_sb, 4, k_sb),
                (wv_sb, bv_sb, 4, v_sb),
            ):
                for chv in range(16):          # 16 chunks of 8 rows
                    ps = psc.tile([m, 8, 48], F32, tag="cps")
                    for t in range(9):
                        dy, dx = t // 3, t % 3
                        rhs = xp_sb[:, 8 * chv + dy : 8 * chv + dy + 8, dx : dx + 48]
                        nc.tensor.matmul(
                            ps[:, :, :], w_sb[:, t, 0:m], rhs,
                            start=(t == 0), stop=(t == 8),
                        )
                    nc.vector.tensor_scalar_add(
                        dst[:, 8 * chv : 8 * chv + 8, :], ps[:, :, :],
                        b_sb[0:m, 0:1],
                    )

        # ---- 3-variant phi-shifted staging in DRAM ----
        zero_sb = P1.tile([3, 2304], F32, tag="zeros")
        nc.vector.memset(zero_sb, 0.0)
        kp_drs, vp_drs = [], []
        for v, phi in enumerate(PHIS):
            for src_sb, lst, nm in ((k_sb, kp_drs, "kp"), (v_sb, vp_drs, "vp")):
                buf = dram.tile([3, 144 * 64], F32, tag=f"{nm}{v}")
                for t in range(4):
                    nc.sync.dma_start(
                        out=buf[:, 2304 * t : 2304 * (t + 1)], in_=zero_sb
                    )
                bv_ = buf[:, :].rearrange("p (a b) -> p a b", a=144)
                if phi == 0:
                    nc.gpsimd.dma_start(
                        out=bv_[:, 8:136, 8:56], in_=src_sb[0:3, :, :]
                    )
                else:
                    nc.gpsimd.dma_start(
                        out=bv_[:, 0 : 136 - phi, 8:56],
                        in_=src_sb[0:3, phi - 8 : 128, :],
                    )
                    nc.gpsimd.dma_start(
                        out=bv_[:, 152 - phi : 144, 8:56],
                        in_=src_sb[1:4, 0 : phi - 8, :],
                    )
                lst.append(buf)

        # ---- static gathers: uk_stack/uv_stack [12, 144, 40] per window ----
        uk_rep = []
        uvT = []
        for j in range(2):
            ukr = P.tile([12, M0, M1], F32R, tag=f"ukrep{j}")
            uvstk = P1.tile([12, M0, M1], F32R, tag="uvstack")
            for v in range(3):
                for buf, dst in ((kp_drs[v], ukr), (vp_drs[v], uvstk)):
                    src = AP(
                        tensor=buf.tensor,
                        offset=buf.offset + 24 * j,
                        ap=[[6144, 4], [48, M0], [1, M1]],
                    )
                    nc.gpsimd.dma_start(
                        out=dst[4 * v : 4 * v + 4, :, :], in_=src.bitcast(F32R)
                    )
            uk_rep.append(ukr)

            # one-hot select of the active variant's uv: [4, 5760]
            uvs = P1.tile([4, M0, M1], F32, tag="uvs")
            uvs_f = uvs.rearrange("p a b -> p (a b)")
            uvstk_f = uvstk[:, :, :].rearrange("p a b -> p (a b)")
            with tc.tile_pool(name=f"pssel{j}", bufs=2, space="PSUM") as pssel:
                for t in range(12):
                    pss = pssel.tile([4, 480], F32, tag="pss")
                    nc.tensor.matmul(
                        pss, sel_sb[:, :], uvstk_f[:, 480 * t : 480 * (t + 1)],
                        start=True, stop=True,
                    )
                    nc.vector.tensor_copy(uvs_f[:, 480 * t : 480 * (t + 1)], pss)

            # transpose uv chunks to [120, 48, 5]; col 4 = ones
            uvt = P.tile([KC, NKC, 5], F32R, tag=f"uvt{j}")
            ones_b = AP(tensor=concol.tensor, offset=concol[0:KC, 1:2].offset,
                        ap=[concol[0:KC, 1:2].ap[0], [0, NKC]])
            nc.vector.tensor_copy(uvt[:, :, 0], ones_b)
            with tc.tile_pool(name=f"pst{j}", bufs=2, space="PSUM") as pst:
                for c in range(NKC):
                    tp = pst.tile([KC, 4], F32, tag="tp")
                    nc.tensor.transpose(
                        tp, uvs_f[:, KC * c : KC * (c + 1)], ident[:, :]
                    )
                    nc.vector.tensor_copy(uvt[:, c, 1:5], tp)
            uvT.append(uvt)

        if debug:
            nc.sync.dma_start(out=dbg['q'][:, :], in_=q_sb[:, :, :].rearrange("p a b -> p (a b)").bitcast(F32))
            nc.sync.dma_start(out=dbg['k'][:, :], in_=k_sb[:, :, :].rearrange("p a b -> p (a b)"))
            nc.sync.dma_start(out=dbg['uk0'][:, :], in_=uk_rep[0][:, :, :].rearrange("p a b -> p (a b)").bitcast(F32))
            nc.sync.dma_start(out=dbg['uvt0'][:, :], in_=uvT[0][:, :, :].rearrange("p a b -> p (a b)").bitcast(F32))

        ctx1.close()  # free phase1 SBUF
        PL = ctx.enter_context(tc.tile_pool(name="late", bufs=1))

        # ---- attention ----
        o_pad = PL.tile([5, 130, 50], F32R, tag="opad")
        zero_b = AP(tensor=concol.tensor, offset=concol[0:5, 0:1].offset,
                    ap=[concol[0:5, 0:1].ap[0], [0, 130 * 50]])
        nc.vector.tensor_copy(o_pad[:, :, :].rearrange("p a b -> p (a b)"), zero_b)

        with (
            tc.tile_pool(name="psqk", bufs=2, space="PSUM") as psqk,
            tc.tile_pool(name="psav", bufs=1, space="PSUM") as psav,
            tc.tile_pool(name="psden", bufs=1, space="PSUM") as psden,
            tc.tile_pool(name="expp", bufs=2) as expp,
            tc.tile_pool(name="redp", bufs=2) as redp,
        ):
            for j in range(2):
                uk_f = uk_rep[j][:, :, :].rearrange("p a b -> p (a b)")
                for qc in range(NQC):
                    ps_av = psav.tile([128, QC], F32, tag="av")
                    for rnd in range(NKC // RPB):       # 8 rounds of 6 chunks
                        ps_qk = psqk.tile([128, RPB, 512], F32, tag="qk")
                        for b in range(RPB):
                            c = RPB * rnd + b           # kv chunk id
                            rhs = q_sb[
                                :,
                                QROWS * qc : QROWS * (qc + 1),
                                24 * j : 24 * j + 24,
                            ]
                            out = ps_qk[0:KC, b, 0:QC].rearrange(
                                "p (a c) -> p a c", a=QROWS
                            )
                            nc.tensor.matmul(
                                out,
                                uk_f[:, KC * c : KC * (c + 1)],
                                rhs,
                                start=True, stop=True,
                            )
                        ex = expp.tile([KC, RPB, QC], F32R, tag="ex")
                        nc.scalar.activation(
                            ex, ps_qk[0:KC, :, 0:QC],
                            mybir.ActivationFunctionType.Exp,
                        )
                        for b in range(RPB):
                            c = RPB * rnd + b
                            nc.tensor.matmul(
                                ps_av[0:5, :],
                                uvT[j][:, c, :],
                                ex[:, b, :],
                                start=(c == 0), stop=(c == NKC - 1),
                            )
                    # normalize and write into o_pad interior
                    s0 = redp.tile([5, QC], F32, tag="s0")
                    nc.vector.tensor_copy(s0, ps_av[0:5, :])
                    rec1 = redp.tile([1, QC], F32, tag="rec1")
                    nc.vector.reciprocal(rec1, s0[0:1, :])
                    rec1r = redp.tile([1, QC], F32R, tag="rec1r")
                    nc.vector.tensor_copy(rec1r, rec1)
                    ps_den = psden.tile([5, QC], F32, tag="den")
                    nc.tensor.matmul(ps_den, ones14[:, :], rec1r,
                                     start=True, stop=True)
                    o_div = redp.tile([5, QC], F32, tag="odiv")
                    nc.vector.tensor_tensor(out=o_div, in0=s0[:, :],
                                            in1=ps_den,
                                            op=mybir.AluOpType.mult)
                    dst = o_pad[
                        :, 1 + QROWS * qc : 1 + QROWS * (qc + 1), 1 + 24 * j : 25 + 24 * j
                    ]
                    nc.vector.tensor_copy(
                        dst, o_div.rearrange("p (a c) -> p a c", a=QROWS)
                    )

        if debug:
            nc.sync.dma_start(out=dbg['opad'][:, :], in_=o_pad[:, :, :].rearrange("p a b -> p (a b)").bitcast(F32))

        # ---- final conv ----
        out_sb = PL.tile([COUT, H, W], F32, tag="outsb")
        with tc.tile_pool(name="psf", bufs=4, space="PSUM") as psf:
            for chv in range(16):
                ps = psf.tile([COUT, 8, 48], F32, tag="fps")
                for t in range(9):
                    dy, dx = t // 3, t % 3
                    rhs = o_pad[:, 8 * chv + dy : 8 * chv + dy + 8, dx : dx + 48]
                    nc.tensor.matmul(
                        ps[:, :, :], wo_sb[:, t, :], rhs,
                        start=(t == 0), stop=(t == 8),
                    )
                nc.vector.tensor_copy(out_sb[:, 8 * chv : 8 * chv + 8, :], ps)
        nc.sync.dma_start(
            out=out_d[:, :], in_=out_sb.rearrange("p a b -> p (a b)")
        )

    nc.compile()
    return nc


_NC = None


def _get_nc():
    global _NC
    if _NC is None:
        _NC = build_nc()
    return _NC


def make_in_maps(x, wq, bq, wk, bk, wv, bv, wo):
    x = np.asarray(x, np.float32)[0]           # [64, 128, 48]
    xp = np.zeros((CIN, 130, 50), np.float32)
    xp[:, 1:129, 1:49] = x
    xp = xp.reshape(CIN, -1)
    s = np.float32(DPH ** -0.5)

    def taps(w):                                # [O, I, 3, 3] -> [I, 9, O]
        return np.ascontiguousarray(np.transpose(w, (1, 2, 3, 0)).reshape(
            w.shape[1], 9, w.shape[0]))

    wq_np = np.asarray(wq, np.float32)
    wk_np = np.asarray(wk, np.float32) * s
    wv_np = np.asarray(wv, np.float32)
    wo_np = np.asarray(wo, np.float32)
    bq_np = np.asarray(bq, np.float32)
    bk_np = np.asarray(bk, np.float32) * s
    bv_np = np.asarray(bv, np.float32)

    in_maps = []
    for h in range(8):
        c_lo = (24576 * h) // 9216
        phi = (24576 * h - 9216 * c_lo) // 64
        v_idx = PHIS.index(phi)

        wq_stack = np.zeros((12, CIN, 3, 3), np.float32)
        wq_stack[4 * v_idx : 4 * v_idx + 4] = wq_np[4 * h : 4 * h + 4]
        bq_stack = np.zeros((12,), np.float32)
        bq_stack[4 * v_idx : 4 * v_idx + 4] = bq_np[4 * h : 4 * h + 4]
        sel = np.zeros((12, 4), np.float32)
        sel[4 * v_idx : 4 * v_idx + 4] = np.eye(4, dtype=np.float32)

        wo_t4 = np.ascontiguousarray(
            np.transpose(wo_np[:, 4 * h : 4 * h + 4], (1, 2, 3, 0))
        ).reshape(4, -1)
        wo_t = np.concatenate([np.zeros((1, wo_t4.shape[1]), np.float32),
                               wo_t4], axis=0)
        in_maps.append({
            "xp": xp,
            "wq_t": taps(wq_stack).reshape(CIN, -1),
            "wk_t": taps(wk_np[c_lo : c_lo + 4]).reshape(CIN, -1),
            "wv_t": taps(wv_np[c_lo : c_lo + 4]).reshape(CIN, -1),
            "wo_t": wo_t,
            "bq_l": bq_stack.reshape(12, 1),
            "bk_l": bk_np[c_lo : c_lo + 4].reshape(4, 1),
            "bv_l": bv_np[c_lo : c_lo + 4].reshape(4, 1),
            "sel": sel,
            "ident4": np.eye(4, dtype=np.float32),
            "concol": np.stack([np.zeros(128, np.float32),
                                np.ones(128, np.float32)], axis=1),
            "ones14": np.ones((1, 5), np.float32),
        })
    return in_maps


def kernel(x, wq, bq, wk, bk, wv, bv, wo):
    from concourse.bass_utils import run_bass_kernel_spmd

    nc = _get_nc()
    in_maps = make_in_maps(x, wq, bq, wk, bk, wv, bv, wo)
    res = run_bass_kernel_spmd(nc, in_maps, list(range(8))).results
    out = np.zeros((COUT, H * W), np.float32)
    for m in res:
        out = out + m["out"]
    return out.reshape(1, COUT, H, W)



# revision 35
# speedup vs baseline: 1.8396x; 1.8396x over previous
"""Trainium2 Bass kernel for nn_MultiHeadAttention_75737453297867.

Sharding: one head per NeuronCore (8 heads / 8 cores). The reference's
aliased as_strided gather needs a per-core base offset 24576*h into the
flange-padded k/v storage; 24576*h mod 9216 is row-aligned (phi in
{0,48,96}), so three phi-shifted staging variants are built statically
and selection happens through host data alone: q and v convs run with
12 stacked channels (inactive variants host-zeroed), the QK contraction
runs over K=12, AV produces 13 rows (1 denominator + 12 variant-dph)
and the final conv weights (host-built) pick the active variant.

Staging path (cost model: DMA time = free bytes/partition * 0.386ns):
conv out [28,128,48] -> PE-transposed to row-major [128,16,48] ->
per-variant phi-shifted writes into flat DRAM ([144,64]-padded channel
images) -> one flat [12,9216] load back for uk (contiguous rows, no 2x
small-elem penalty) + direct strided DMAs building uvT [120,13,48].
QK reads uk chunks via strided APs (row-aliased windows), exp on ACT
from PSUM, AV accumulates [13,384] on PE, normalize writes straight
into the padded o image, final 3x3 conv, partial outputs summed on
host.
"""

import sys

import numpy as np

if "/opt/trn_rl_repo" not in sys.path:
    sys.path.insert(0, "/opt/trn_rl_repo")

import concourse.bass as bass
import concourse.tile as tile
from concourse import bacc
from concourse import mybir
from concourse.bass_types import AP

# Problem constants
CIN, COUT, H, W = 64, 64, 128, 48
DM, NH, DPH = 32, 8, 4
Q0, Q1, F0, F1 = 128, 24, 8, 8
M0, M1 = Q0 + 2 * F0, Q1 + 2 * F1          # 144, 40
KV = M0 * M1                               # 5760
CH = 144 * 64                              # 9216 flat padded-channel size
DST = 6144                                 # d-stride (Hp*Wp) in flat coords
KC = 120                                   # kv chunk = 3 m0-rows x 40
NKC = 48
RPB = 3                                    # kv chunks per psum round
NQC = 8                                    # q chunks of 384 (16 rows x 24)
QC = 384
QROWS = 16
PHIS = (0, 48, 96)
F32 = mybir.dt.float32
F32R = mybir.dt.float32r


def build_nc():
    nc = bacc.Bacc()

    xp_d = nc.dram_tensor("xp", [CIN, 130 * 50], F32R, kind="ExternalInput")
    wqkv_d = nc.dram_tensor("wqkv_t", [CIN, 9 * 48], F32R, kind="ExternalInput")
    b48_d = nc.dram_tensor("b48", [48, 1], F32, kind="ExternalInput")
    wo_d = nc.dram_tensor("wo_t", [13, 9 * 64], F32R, kind="ExternalInput")
    id16_d = nc.dram_tensor("id16", [16, 16], F32, kind="ExternalInput")
    id12_d = nc.dram_tensor("id12", [12, 12], F32R, kind="ExternalInput")
    o13_d = nc.dram_tensor("ones13", [1, 13], F32R, kind="ExternalInput")
    out_d = nc.dram_tensor("out", [COUT, H * W], F32, kind="ExternalOutput")

    from contextlib import ExitStack

    with tile.TileContext(nc) as tc, ExitStack() as ctx:
        P = ctx.enter_context(tc.tile_pool(name="persist", bufs=1))
        dram = ctx.enter_context(tc.tile_pool(name="dram", bufs=1, space="DRAM"))
        ctx1 = ctx.enter_context(ExitStack())
        P1 = ctx1.enter_context(tc.tile_pool(name="phase1", bufs=1))

        # ---- input loads (xp split across two DMA lanes) ----
        xp_sb = P1.tile([CIN, 130, 50], F32R, tag="xp")
        nc.sync.dma_start(
            out=xp_sb[:, 0:65, :],
            in_=xp_d[:, 0:3250].rearrange("p (a b) -> p a b", a=65),
        )
        nc.gpsimd.dma_start(
            out=xp_sb[:, 65:130, :],
            in_=xp_d[:, 3250:6500].rearrange("p (a b) -> p a b", a=65),
        )
        wqkv_sb = P.tile([CIN, 9, 48], F32R, tag="wqkv")
        nc.sync.dma_start(
            out=wqkv_sb, in_=wqkv_d[:, :].rearrange("p (t o) -> p t o", t=9)
        )
        b48 = P.tile([48, 1], F32, tag="b48")
        nc.sync.dma_start(out=b48, in_=b48_d[:, :])
        wo_sb = P.tile([13, 9, 64], F32R, tag="wo")
        nc.sync.dma_start(
            out=wo_sb, in_=wo_d[:, :].rearrange("p (t o) -> p t o", t=9)
        )
        id16 = P.tile([16, 16], F32, tag="id16")
        nc.sync.dma_start(out=id16, in_=id16_d[:, :])
        id12 = P.tile([12, 12], F32R, tag="id12")
        nc.sync.dma_start(out=id12, in_=id12_d[:, :])
        ones13 = P.tile([1, 13], F32R, tag="ones13")
        nc.sync.dma_start(out=ones13, in_=o13_d[:, :])

        zero_sb = P1.tile([128, 648], F32, tag="zeros")
        nc.vector.memset(zero_sb, 0.0)

        # ---- DRAM staging buffers (3 variants x 3 channels each) ----
        kp_all = dram.tile([9, CH], F32, tag="kp")
        vp_all = dram.tile([9, CH], F32, tag="vp")
        for buf in (kp_all, vp_all):
            dst = AP(tensor=buf.tensor, offset=buf.offset,
                     ap=[[648, 128], [1, 648]])
            nc.scalar.dma_start(out=dst, in_=zero_sb[:, :])

        # ---- stacked q/k/v conv: rows 0-11 = q, rows 32-47 = k4 + v12 ----
        # (kv starts at 32: engine PSUM access must be 32-partition aligned)
        q_sb = P.tile([12, 128, 48], F32R, tag="q_sb")
        kv_sb = P1.tile([16, 128, 48], F32, tag="kv_sb")
        with tc.tile_pool(name="psc", bufs=4, space="PSUM") as psc:
            for chv in range(16):
                ps = psc.tile([48, 8, 48], F32, tag="cps")
                for t in range(9):
                    dy, dx = t // 3, t % 3
                    rhs = xp_sb[:, 8 * chv + dy : 8 * chv + dy + 8, dx : dx + 48]
                    nc.tensor.matmul(
                        ps[:, :, :], wqkv_sb[:, t, 0:48], rhs,
                        start=(t == 0), stop=(t == 8),
                    )
                nc.vector.tensor_scalar_add(
                    q_sb[:, 8 * chv : 8 * chv + 8, :], ps[0:12, :, :],
                    b48[0:12, 0:1],
                )
                nc.vector.tensor_scalar_add(
                    kv_sb[:, 8 * chv : 8 * chv + 8, :], ps[32:48, :, :],
                    b48[32:48, 0:1],
                )

        # ---- transpose k/v to row-major [128 rows, 16 ch, 48 cols] ----
        kv_row = P1.tile([128, 16, 48], F32, tag="kv_row")
        with tc.tile_pool(name="pst", bufs=2, space="PSUM") as pst:
            for x in range(48):
                tp = pst.tile([128, 16], F32, tag="tp")
                nc.tensor.matmul(tp, kv_sb[:, :, x], id16[:, :],
                                 start=True, stop=True)
                nc.vector.tensor_copy(kv_row[:, :, x], tp)

        # ---- phi-shifted staging writes into the padded channel images ----
        engs = [nc.sync, nc.gpsimd, nc.scalar]
        ei = 0
        for buf_all, cbase in ((kp_all, lambda v: 0), (vp_all, lambda v: 4 + 4 * v)):
            for v, phi in enumerate(PHIS):
                cb = cbase(v)
                base = buf_all.offset + 3 * v * CH
                if phi == 0:
                    dst = AP(tensor=buf_all.tensor, offset=base + 8 * 64 + 8,
                             ap=[[64, 128], [CH, 3], [1, 48]])
                    engs[ei % 3].dma_start(out=dst, in_=kv_row[0:128, cb : cb + 3, :])
                    ei += 1
                else:
                    n1 = 136 - phi
                    dst1 = AP(tensor=buf_all.tensor, offset=base + 8,
                              ap=[[64, n1], [CH, 3], [1, 48]])
                    engs[ei % 3].dma_start(
                        out=dst1, in_=kv_row[phi - 8 : 128, cb : cb + 3, :])
                    ei += 1
                    n2 = phi - 8
                    dst2 = AP(tensor=buf_all.tensor,
                              offset=base + (152 - phi) * 64 + 8,
                              ap=[[64, n2], [CH, 3], [1, 48]])
                    engs[ei % 3].dma_start(
                        out=dst2, in_=kv_row[0 : phi - 8, cb + 1 : cb + 4, :])
                    ei += 1

        # ---- padded attention-output image; zero only the 1-px border ----
        o_pad = P.tile([13, 130, 50], F32R, tag="opad")
        zb = zero_sb[0:13, 0:1]
        for dst in (o_pad[:, 0, :], o_pad[:, 129, :],
                    o_pad[:, 1:129, 0], o_pad[:, 1:129, 49]):
            n = dst.free_size()
            src = AP(tensor=zb.tensor, offset=zb.offset, ap=[zb.ap[0], [0, n]])
            nc.vector.tensor_copy(dst, src)

        ctx1.close()  # free xp / kv_sb / kv_row / zeros SBUF
        ctx2 = ctx.enter_context(ExitStack())
        P2 = ctx2.enter_context(tc.tile_pool(name="phase2", bufs=1))
        uvp = ctx2.enter_context(tc.tile_pool(name="uvp", bufs=1))

        # ---- flat uk load: partition (v,d) = contiguous 9216-elem span ----
        uk_flat = P2.tile([12, CH], F32R, tag="uk")
        for i, eng in enumerate((nc.sync, nc.gpsimd, nc.scalar)):
            lo, hi = 3072 * i, 3072 * (i + 1)
            src = AP(tensor=kp_all.tensor, offset=kp_all.offset + lo,
                     ap=[[3 * CH, 3], [DST, 4], [1, hi - lo]])
            eng.dma_start(out=uk_flat[:, lo:hi], in_=src.bitcast(F32R))

        # ---- flat uv load ----
        uv_flat = P2.tile([12, CH], F32R, tag="uv")
        for i, eng in enumerate((nc.scalar, nc.sync, nc.gpsimd)):
            lo, hi = 3072 * i, 3072 * (i + 1)
            src = AP(tensor=vp_all.tensor, offset=vp_all.offset + lo,
                     ap=[[3 * CH, 3], [DST, 4], [1, hi - lo]])
            eng.dma_start(out=uv_flat[:, lo:hi], in_=src.bitcast(F32R))

        # ---- materialize flat window views (stationary matmul operands
        # must be 1-free-dim): engine copies with strided window reads;
        # uvT [120, 13, 48]: slot 0 = ones (denominator), slots 1-12 =
        # PE-transposed [12,120] kv chunks ----
        ukb, uvb = uk_flat[:, 0:1], uv_flat[:, 0:1]
        id12r = id12[:, :]
        uk_rep, uvT = [], []
        with tc.tile_pool(name="pstv", bufs=4, space="PSUM") as pstv:
            for j in range(2):
                ukr = P.tile([12, KV], F32R, tag=f"ukr{j}")
                src = AP(tensor=ukb.tensor, offset=ukb.offset + 24 * j,
                         ap=[ukb.ap[0], [48, M0], [1, M1]])
                if j == 0:
                    nc.scalar.copy(ukr[:, :], src)
                else:
                    nc.gpsimd.tensor_copy(ukr[:, :], src)
                uk_rep.append(ukr)

                uvr = uvp.tile([12, KV], F32R, tag="uvr")
                src = AP(tensor=uvb.tensor, offset=uvb.offset + 24 * j,
                         ap=[uvb.ap[0], [48, M0], [1, M1]])
                if j == 0:
                    nc.vector.tensor_copy(uvr[:, :], src)
                else:
                    nc.scalar.copy(uvr[:, :], src)

                t = P.tile([KC, 13, 48], F32R, tag=f"uvt{j}")
                nc.vector.memset(t[:, 0, :].bitcast(F32), 1.0)
                uvT.append(t)
                for c in range(NKC):
                    tp = pstv.tile([KC, 12], F32R, tag="tpv")
                    nc.tensor.transpose(
                        tp, uvr[:, KC * c : KC * (c + 1)], id12r)
                    if c % 2:
                        nc.vector.tensor_copy(uvT[j][:, 1:13, c], tp)
                    else:
                        nc.scalar.copy(uvT[j][:, 1:13, c], tp)

        ctx2.close()  # free uk_flat / uv_flat / uvr SBUF
        PL = ctx.enter_context(tc.tile_pool(name="late", bufs=1))

        # ---- attention ----
        with (
            tc.tile_pool(name="psqk", bufs=2, space="PSUM") as psqk,
            tc.tile_pool(name="psav", bufs=1, space="PSUM") as psav,
            tc.tile_pool(name="psden", bufs=1, space="PSUM") as psden,
            tc.tile_pool(name="expp", bufs=2) as expp,
            tc.tile_pool(name="redp", bufs=2) as redp,
        ):
            for j in range(2):
                for qc in range(NQC):
                    ps_av = psav.tile([13, QC], F32, tag="av")
                    for rnd in range(NKC // RPB):
                        ps_qk = psqk.tile([KC, RPB, 512], F32, tag="qk")
                        for b in range(RPB):
                            c = RPB * rnd + b
                            lhsT = uk_rep[j][:, KC * c : KC * (c + 1)]
                            rhs = q_sb[
                                :,
                                QROWS * qc : QROWS * (qc + 1),
                                24 * j : 24 * j + 24,
                            ]
                            out = ps_qk[0:KC, b, 0:QC].rearrange(
                                "p (a c) -> p a c", a=QROWS
                            )
                            nc.tensor.matmul(out, lhsT, rhs,
                                             start=True, stop=True)
                        ex = expp.tile([KC, RPB, QC], F32R, tag="ex")
                        nc.scalar.activation(
                            ex, ps_qk[0:KC, :, 0:QC],
                            mybir.ActivationFunctionType.Exp,
                        )
                        for b in range(RPB):
                            c = RPB * rnd + b
                            nc.tensor.matmul(
                                ps_av[:, :], uvT[j][:, :, c], ex[:, b, :],
                                start=(c == 0), stop=(c == NKC - 1),
                            )
                    # normalize: row 0 of ps_av is the softmax denominator
                    s0 = redp.tile([13, QC], F32, tag="s0")
                    nc.vector.tensor_copy(s0, ps_av[:, :])
                    rec1 = redp.tile([1, QC], F32, tag="rec")
                    nc.vector.reciprocal(rec1, s0[0:1, :])
                    rec1r = redp.tile([1, QC], F32R, tag="recr")
                    nc.vector.tensor_copy(rec1r, rec1)
                    ps_den = psden.tile([13, QC], F32, tag="den")
                    nc.tensor.matmul(ps_den, ones13[:, :], rec1r,
                                     start=True, stop=True)
                    o_div = redp.tile([13, QC], F32, tag="odiv")
                    nc.vector.tensor_tensor(out=o_div, in0=s0[:, :],
                                            in1=ps_den,
                                            op=mybir.AluOpType.mult)
                    dst = o_pad[
                        :,
                        1 + QROWS * qc : 1 + QROWS * (qc + 1),
                        1 + 24 * j : 25 + 24 * j,
                    ]
                    nc.vector.tensor_copy(
                        dst, o_div.rearrange("p (a c) -> p a c", a=QROWS)
                    )

        # ---- final conv ----
        out_sb = PL.tile([COUT, 128, 48], F32, tag="outsb")
        with tc.tile_pool(name="psf", bufs=4, space="PSUM") as psf:
            for chv in range(16):
                ps = psf.tile([COUT, 8, 48], F32, tag="fps")
                for t in range(9):
                    dy, dx = t // 3, t % 3
                    rhs = o_pad[
                        :, 8 * chv + dy : 8 * chv + dy + 8, dx : dx + 48
                    ].bitcast(F32R)
                    nc.tensor.matmul(ps[:, :, :], wo_sb[:, t, :], rhs,
                                     start=(t == 0), stop=(t == 8))
                nc.vector.tensor_copy(out_sb[:, 8 * chv : 8 * chv + 8, :], ps)
        nc.sync.dma_start(
            out=out_d[:, 0:3072],
            in_=out_sb[:, 0:64, :].rearrange("p a b -> p (a b)"),
        )
        nc.gpsimd.dma_start(
            out=out_d[:, 3072:6144],
            in_=out_sb[:, 64:128, :].rearrange("p a b -> p (a b)"),
        )

    nc.compile()
    return nc


_NC = None


def _get_nc():
    global _NC
    if _NC is None:
        _NC = build_nc()
    return _NC


def make_in_maps(x, wq, bq, wk, bk, wv, bv, wo):
    x = np.asarray(x, np.float32)[0]           # [64, 128, 48]
    xp = np.zeros((CIN, 130, 50), np.float32)
    xp[:, 1:129, 1:49] = x
    xp = xp.reshape(CIN, -1)
    s = np.float32(DPH ** -0.5)

    def taps(w):                                # [O, I, 3, 3] -> [I, 9, O]
        return np.ascontiguousarray(np.transpose(w, (1, 2, 3, 0)).reshape(
            w.shape[1], 9, w.shape[0]))

    wq_np = np.asarray(wq, np.float32)
    wk_np = np.asarray(wk, np.float32) * s
    wv_np = np.asarray(wv, np.float32)
    wo_np = np.asarray(wo, np.float32)
    bq_np = np.asarray(bq, np.float32)
    bk_np = np.asarray(bk, np.float32) * s
    bv_np = np.asarray(bv, np.float32)

    in_maps = []
    for h in range(8):
        c_lo = (24576 * h) // 9216
        phi = (24576 * h - 9216 * c_lo) // 64
        v_idx = PHIS.index(phi)

        wqkv = np.zeros((48, CIN, 3, 3), np.float32)
        wqkv[4 * v_idx : 4 * v_idx + 4] = wq_np[4 * h : 4 * h + 4]
        wqkv[32:36] = wk_np[c_lo : c_lo + 4]
        wqkv[36 + 4 * v_idx : 36 + 4 * v_idx + 4] = wv_np[c_lo : c_lo + 4]

        b48 = np.zeros((48,), np.float32)
        b48[4 * v_idx : 4 * v_idx + 4] = bq_np[4 * h : 4 * h + 4]
        b48[32:36] = bk_np[c_lo : c_lo + 4]
        b48[36 + 4 * v_idx : 36 + 4 * v_idx + 4] = bv_np[c_lo : c_lo + 4]

        wo_t4 = np.ascontiguousarray(
            np.transpose(wo_np[:, 4 * h : 4 * h + 4], (1, 2, 3, 0))
        ).reshape(4, -1)
        wo13 = np.zeros((13, wo_t4.shape[1]), np.float32)
        wo13[1 + 4 * v_idx : 1 + 4 * v_idx + 4] = wo_t4

        in_maps.append({
            "xp": xp,
            "wqkv_t": taps(wqkv).reshape(CIN, -1),
            "b48": b48.reshape(48, 1),
            "wo_t": wo13,
            "id16": np.eye(16, dtype=np.float32),
            "id12": np.eye(12, dtype=np.float32),
            "ones13": np.ones((1, 13), np.float32),
        })
    return in_maps


def kernel(x, wq, bq, wk, bk, wv, bv, wo):
    from concourse.bass_utils import run_bass_kernel_spmd

    nc = _get_nc()
    in_maps = make_in_maps(x, wq, bq, wk, bk, wv, bv, wo)
    res = run_bass_kernel_spmd(nc, in_maps, list(range(8))).results
    out = np.zeros((COUT, H * W), np.float32)
    for m in res:
        out = out + m["out"]
    return out.reshape(1, COUT, H, W)


# revision 40
# speedup vs baseline: 1.9272x; 1.0476x over previous
"""Trainium2 Bass kernel for nn_MultiHeadAttention_75737453297867.

Sharding: one head per NeuronCore (8 heads / 8 cores). The reference's
aliased as_strided gather needs a per-core base offset 24576*h into the
flange-padded k/v storage; 24576*h mod 9216 is row-aligned (phi in
{0,48,96}), so three phi-shifted staging variants are built statically
and selection happens through host data alone: q and v convs run with
12 stacked channels (inactive variants host-zeroed), the QK contraction
runs over K=12, AV produces 13 rows (1 denominator + 12 variant-dph)
and the final conv weights (host-built) pick the active variant.

Staging path (cost model: DMA time = free bytes/partition * 0.386ns):
conv out [28,128,48] -> PE-transposed to row-major [128,16,48] ->
per-variant phi-shifted writes into flat DRAM ([144,64]-padded channel
images) -> one flat [12,9216] load back for uk (contiguous rows, no 2x
small-elem penalty) + direct strided DMAs building uvT [120,13,48].
QK reads uk chunks via strided APs (row-aliased windows), exp on ACT
from PSUM, AV accumulates [13,384] on PE, normalize writes straight
into the padded o image, final 3x3 conv, partial outputs summed on
host.
"""

import sys

import numpy as np

if "/opt/trn_rl_repo" not in sys.path:
    sys.path.insert(0, "/opt/trn_rl_repo")

import concourse.bass as bass
import concourse.tile as tile
from concourse import bacc
from concourse import mybir
from concourse.bass_types import AP

# Problem constants
CIN, COUT, H, W = 64, 64, 128, 48
DM, NH, DPH = 32, 8, 4
Q0, Q1, F0, F1 = 128, 24, 8, 8
M0, M1 = Q0 + 2 * F0, Q1 + 2 * F1          # 144, 40
KV = M0 * M1                               # 5760
CH = 144 * 64                              # 9216 flat padded-channel size
DST = 6144                                 # d-stride (Hp*Wp) in flat coords
KC = 128                                   # kv chunk: flat 128-slice of window
NKC = 45
RPB = 3                                    # kv chunks per psum round
NQC = 8                                    # q chunks of 384 (16 rows x 24)
QC = 384
QROWS = 16
PHIS = (0, 48, 96)
F32 = mybir.dt.float32
F32R = mybir.dt.float32r


def build_nc():
    nc = bacc.Bacc()

    xp_d = nc.dram_tensor("xp", [CIN, 130 * 50], F32R, kind="ExternalInput")
    wqkv_d = nc.dram_tensor("wqkv_t", [CIN, 9 * 48], F32R, kind="ExternalInput")
    b48_d = nc.dram_tensor("b48", [48, 1], F32, kind="ExternalInput")
    wo_d = nc.dram_tensor("wo_t", [13, 9 * 64], F32R, kind="ExternalInput")
    id16_d = nc.dram_tensor("id16", [16, 16], F32, kind="ExternalInput")
    id12_d = nc.dram_tensor("id12", [12, 12], F32R, kind="ExternalInput")
    o13_d = nc.dram_tensor("ones13", [1, 13], F32R, kind="ExternalInput")
    out_d = nc.dram_tensor("out", [COUT, H * W], F32, kind="ExternalOutput")

    from contextlib import ExitStack

    with tile.TileContext(nc) as tc, ExitStack() as ctx:
        P = ctx.enter_context(tc.tile_pool(name="persist", bufs=1))
        dram = ctx.enter_context(tc.tile_pool(name="dram", bufs=1, space="DRAM"))
        ctx1 = ctx.enter_context(ExitStack())
        P1 = ctx1.enter_context(tc.tile_pool(name="phase1", bufs=1))

        # ---- input loads (xp split across two DMA lanes) ----
        xp_sb = P1.tile([CIN, 130, 50], F32R, tag="xp")
        nc.sync.dma_start(
            out=xp_sb[:, 0:65, :],
            in_=xp_d[:, 0:3250].rearrange("p (a b) -> p a b", a=65),
        )
        nc.gpsimd.dma_start(
            out=xp_sb[:, 65:130, :],
            in_=xp_d[:, 3250:6500].rearrange("p (a b) -> p a b", a=65),
        )
        wqkv_sb = P.tile([CIN, 9, 48], F32R, tag="wqkv")
        nc.sync.dma_start(
            out=wqkv_sb, in_=wqkv_d[:, :].rearrange("p (t o) -> p t o", t=9)
        )
        b48 = P.tile([48, 1], F32, tag="b48")
        nc.sync.dma_start(out=b48, in_=b48_d[:, :])
        wo_sb = P.tile([13, 9, 64], F32R, tag="wo")
        nc.sync.dma_start(
            out=wo_sb, in_=wo_d[:, :].rearrange("p (t o) -> p t o", t=9)
        )
        id16 = P.tile([16, 16], F32, tag="id16")
        nc.sync.dma_start(out=id16, in_=id16_d[:, :])
        id12 = P.tile([12, 12], F32R, tag="id12")
        nc.sync.dma_start(out=id12, in_=id12_d[:, :])
        ones13 = P.tile([1, 13], F32R, tag="ones13")
        nc.sync.dma_start(out=ones13, in_=o13_d[:, :])

        zero_sb = P1.tile([128, 648], F32, tag="zeros")
        nc.vector.memset(zero_sb, 0.0)

        # ---- DRAM staging buffers (3 variants x 3 channels each) ----
        kp_all = dram.tile([9, CH], F32, tag="kp")
        vp_all = dram.tile([9, CH], F32, tag="vp")
        for buf in (kp_all, vp_all):
            dst = AP(tensor=buf.tensor, offset=buf.offset,
                     ap=[[648, 128], [1, 648]])
            nc.scalar.dma_start(out=dst, in_=zero_sb[:, :])

        # ---- stacked q/k/v conv: rows 0-11 = q, rows 32-47 = k4 + v12 ----
        # (kv starts at 32: engine PSUM access must be 32-partition aligned)
        q_sb = P.tile([12, 128, 48], F32R, tag="q_sb")
        kv_sb = P1.tile([16, 128, 48], F32, tag="kv_sb")
        with tc.tile_pool(name="psc", bufs=4, space="PSUM") as psc:
            for chv in range(16):
                ps = psc.tile([48, 8, 48], F32, tag="cps")
                for t in range(9):
                    dy, dx = t // 3, t % 3
                    rhs = xp_sb[:, 8 * chv + dy : 8 * chv + dy + 8, dx : dx + 48]
                    nc.tensor.matmul(
                        ps[:, :, :], wqkv_sb[:, t, 0:48], rhs,
                        start=(t == 0), stop=(t == 8),
                    )
                nc.vector.tensor_scalar_add(
                    q_sb[:, 8 * chv : 8 * chv + 8, :], ps[0:12, :, :],
                    b48[0:12, 0:1],
                )
                nc.vector.tensor_scalar_add(
                    kv_sb[:, 8 * chv : 8 * chv + 8, :], ps[32:48, :, :],
                    b48[32:48, 0:1],
                )

        # ---- transpose k/v to row-major [128 rows, 16 ch, 48 cols] ----
        kv_row = P1.tile([128, 16, 48], F32, tag="kv_row")
        with tc.tile_pool(name="pst", bufs=2, space="PSUM") as pst:
            for x in range(48):
                tp = pst.tile([128, 16], F32, tag="tp")
                nc.tensor.matmul(tp, kv_sb[:, :, x], id16[:, :],
                                 start=True, stop=True)
                nc.vector.tensor_copy(kv_row[:, :, x], tp)

        # ---- phi-shifted staging writes into the padded channel images ----
        engs = [nc.sync, nc.gpsimd, nc.scalar]
        ei = 0
        for buf_all, cbase in ((kp_all, lambda v: 0), (vp_all, lambda v: 4 + 4 * v)):
            for v, phi in enumerate(PHIS):
                cb = cbase(v)
                base = buf_all.offset + 3 * v * CH
                if phi == 0:
                    dst = AP(tensor=buf_all.tensor, offset=base + 8 * 64 + 8,
                             ap=[[64, 128], [CH, 3], [1, 48]])
                    engs[ei % 3].dma_start(out=dst, in_=kv_row[0:128, cb : cb + 3, :])
                    ei += 1
                else:
                    n1 = 136 - phi
                    dst1 = AP(tensor=buf_all.tensor, offset=base + 8,
                              ap=[[64, n1], [CH, 3], [1, 48]])
                    engs[ei % 3].dma_start(
                        out=dst1, in_=kv_row[phi - 8 : 128, cb : cb + 3, :])
                    ei += 1
                    n2 = phi - 8
                    dst2 = AP(tensor=buf_all.tensor,
                              offset=base + (152 - phi) * 64 + 8,
                              ap=[[64, n2], [CH, 3], [1, 48]])
                    engs[ei % 3].dma_start(
                        out=dst2, in_=kv_row[0 : phi - 8, cb + 1 : cb + 4, :])
                    ei += 1

        # ---- padded attention-output image; zero only the 1-px border ----
        o_pad = P.tile([13, 130, 50], F32R, tag="opad")
        zb = zero_sb[0:13, 0:1]
        for dst in (o_pad[:, 0, :], o_pad[:, 129, :],
                    o_pad[:, 1:129, 0], o_pad[:, 1:129, 49]):
            n = dst.free_size()
            src = AP(tensor=zb.tensor, offset=zb.offset, ap=[zb.ap[0], [0, n]])
            nc.vector.tensor_copy(dst, src)

        ctx1.close()  # free xp / kv_sb / kv_row / zeros SBUF
        ctx2 = ctx.enter_context(ExitStack())
        P2 = ctx2.enter_context(tc.tile_pool(name="phase2", bufs=1))
        uvp = ctx2.enter_context(tc.tile_pool(name="uvp", bufs=1))

        # ---- flat uk load: partition (v,d) = contiguous 9216-elem span ----
        uk_flat = P2.tile([12, CH], F32R, tag="uk")
        for i, eng in enumerate((nc.sync, nc.gpsimd, nc.scalar)):
            lo, hi = 3072 * i, 3072 * (i + 1)
            src = AP(tensor=kp_all.tensor, offset=kp_all.offset + lo,
                     ap=[[3 * CH, 3], [DST, 4], [1, hi - lo]])
            eng.dma_start(out=uk_flat[:, lo:hi], in_=src.bitcast(F32R))

        # ---- flat uv load (SP/Pool only; keep ACT free for ukr0 + exp) ----
        uv_flat = P2.tile([12, CH], F32R, tag="uv")
        for i, eng in enumerate((nc.sync, nc.gpsimd)):
            lo, hi = 4608 * i, 4608 * (i + 1)
            src = AP(tensor=vp_all.tensor, offset=vp_all.offset + lo,
                     ap=[[3 * CH, 3], [DST, 4], [1, hi - lo]])
            eng.dma_start(out=uv_flat[:, lo:hi], in_=src.bitcast(F32R))

        # ---- materialize flat window views (stationary matmul operands
        # must be 1-free-dim): engine copies with strided window reads;
        # uvT [120, 13, 48]: slot 0 = ones (denominator), slots 1-12 =
        # PE-transposed [12,120] kv chunks ----
        ukb, uvb = uk_flat[:, 0:1], uv_flat[:, 0:1]
        id12r = id12[:, :]
        uk_rep, uvT = [], []
        with tc.tile_pool(name="pstv", bufs=4, space="PSUM") as pstv:
            for j in range(2):
                ukr = P.tile([12, KV], F32R, tag=f"ukr{j}")
                src = AP(tensor=ukb.tensor, offset=ukb.offset + 24 * j,
                         ap=[ukb.ap[0], [48, M0], [1, M1]])
                if j == 0:
                    nc.scalar.copy(ukr[:, :], src)
                else:
                    nc.gpsimd.tensor_copy(ukr[:, :], src)
                uk_rep.append(ukr)

                uvr = uvp.tile([12, KV], F32R, tag="uvr")
                src = AP(tensor=uvb.tensor, offset=uvb.offset + 24 * j,
                         ap=[uvb.ap[0], [48, M0], [1, M1]])
                if j == 0:
                    nc.vector.tensor_copy(uvr[:, :], src)
                else:
                    nc.gpsimd.tensor_copy(uvr[:, :], src)

                t = P.tile([KC, 13, NKC], F32R, tag=f"uvt{j}")
                nc.vector.memset(t[:, 0, :].bitcast(F32), 1.0)
                uvT.append(t)
                for c in range(NKC):
                    tp = pstv.tile([KC, 12], F32R, tag="tpv")
                    nc.tensor.transpose(
                        tp, uvr[:, KC * c : KC * (c + 1)], id12r)
                    nc.vector.tensor_copy(uvT[j][:, 1:13, c], tp)

        ctx2.close()  # free uk_flat / uv_flat / uvr SBUF
        PL = ctx.enter_context(tc.tile_pool(name="late", bufs=1))

        # ---- attention ----
        with (
            tc.tile_pool(name="psqk", bufs=2, space="PSUM") as psqk,
            tc.tile_pool(name="psav", bufs=1, space="PSUM") as psav,
            tc.tile_pool(name="psden", bufs=1, space="PSUM") as psden,
            tc.tile_pool(name="expp", bufs=2) as expp,
            tc.tile_pool(name="redp", bufs=2) as redp,
        ):
            for j in range(2):
                for qc in range(NQC):
                    ps_av = psav.tile([13, QC], F32, tag="av")
                    for rnd in range(NKC // RPB):
                        ps_qk = psqk.tile([KC, RPB, 512], F32, tag="qk")
                        for b in range(RPB):
                            c = RPB * rnd + b
                            lhsT = uk_rep[j][:, KC * c : KC * (c + 1)]
                            rhs = q_sb[
                                :,
                                QROWS * qc : QROWS * (qc + 1),
                                24 * j : 24 * j + 24,
                            ]
                            out = ps_qk[0:KC, b, 0:QC].rearrange(
                                "p (a c) -> p a c", a=QROWS
                            )
                            nc.tensor.matmul(out, lhsT, rhs,
                                             start=True, stop=True)
                        ex = expp.tile([KC, RPB, QC], F32R, tag="ex")
                        nc.scalar.activation(
                            ex, ps_qk[0:KC, :, 0:QC],
                            mybir.ActivationFunctionType.Exp,
                        )
                        for b in range(RPB):
                            c = RPB * rnd + b
                            nc.tensor.matmul(
                                ps_av[:, :], uvT[j][:, :, c], ex[:, b, :],
                                start=(c == 0), stop=(c == NKC - 1),
                            )
                    # normalize: row 0 of ps_av is the softmax denominator
                    s0 = redp.tile([13, QC], F32, tag="s0")
                    nc.vector.tensor_copy(s0, ps_av[:, :])
                    rec1 = redp.tile([1, QC], F32, tag="rec")
                    nc.vector.reciprocal(rec1, s0[0:1, :])
                    rec1r = redp.tile([1, QC], F32R, tag="recr")
                    nc.vector.tensor_copy(rec1r, rec1)
                    ps_den = psden.tile([13, QC], F32, tag="den")
                    nc.tensor.matmul(ps_den, ones13[:, :], rec1r,
                                     start=True, stop=True)
                    o_div = redp.tile([13, QC], F32, tag="odiv")
                    nc.vector.tensor_tensor(out=o_div, in0=s0[:, :],
                                            in1=ps_den,
                                            op=mybir.AluOpType.mult)
                    dst = o_pad[
                        :,
                        1 + QROWS * qc : 1 + QROWS * (qc + 1),
                        1 + 24 * j : 25 + 24 * j,
                    ]
                    nc.vector.tensor_copy(
                        dst, o_div.rearrange("p (a c) -> p a c", a=QROWS)
                    )

        # ---- final conv ----
        out_sb = PL.tile([COUT, 128, 48], F32, tag="outsb")
        dma_engs = (nc.sync, nc.gpsimd, nc.scalar, nc.sync)
        with tc.tile_pool(name="psf", bufs=4, space="PSUM") as psf:
            for chv in range(16):
                ps = psf.tile([COUT, 8, 48], F32, tag="fps")
                for t in range(9):
                    dy, dx = t // 3, t % 3
                    rhs = o_pad[:, 8 * chv + dy : 8 * chv + dy + 8,
                                dx : dx + 48]
                    nc.tensor.matmul(ps[:, :, :], wo_sb[:, t, :], rhs,
                                     start=(t == 0), stop=(t == 8))
                nc.vector.tensor_copy(out_sb[:, 8 * chv : 8 * chv + 8, :], ps)
                if chv % 4 == 3:  # stream the finished quarter out
                    q4 = chv // 4
                    dma_engs[q4].dma_start(
                        out=out_d[:, 1536 * q4 : 1536 * (q4 + 1)],
                        in_=out_sb[:, 32 * q4 : 32 * (q4 + 1), :].rearrange(
                            "p a b -> p (a b)"),
                    )

    nc.compile()
    return nc


_NC = None


def _get_nc():
    global _NC
    if _NC is None:
        _NC = build_nc()
    return _NC


def make_in_maps(x, wq, bq, wk, bk, wv, bv, wo):
    x = np.asarray(x, np.float32)[0]           # [64, 128, 48]
    xp = np.zeros((CIN, 130, 50), np.float32)
    xp[:, 1:129, 1:49] = x
    xp = xp.reshape(CIN, -1)
    s = np.float32(DPH ** -0.5)

    def taps(w):                                # [O, I, 3, 3] -> [I, 9, O]
        return np.ascontiguousarray(np.transpose(w, (1, 2, 3, 0)).reshape(
            w.shape[1], 9, w.shape[0]))

    wq_np = np.asarray(wq, np.float32)
    wk_np = np.asarray(wk, np.float32) * s
    wv_np = np.asarray(wv, np.float32)
    wo_np = np.asarray(wo, np.float32)
    bq_np = np.asarray(bq, np.float32)
    bk_np = np.asarray(bk, np.float32) * s
    bv_np = np.asarray(bv, np.float32)

    in_maps = []
    for h in range(8):
        c_lo = (24576 * h) // 9216
        phi = (24576 * h - 9216 * c_lo) // 64
        v_idx = PHIS.index(phi)

        wqkv = np.zeros((48, CIN, 3, 3), np.float32)
        wqkv[4 * v_idx : 4 * v_idx + 4] = wq_np[4 * h : 4 * h + 4]
        wqkv[32:36] = wk_np[c_lo : c_lo + 4]
        wqkv[36 + 4 * v_idx : 36 + 4 * v_idx + 4] = wv_np[c_lo : c_lo + 4]

        b48 = np.zeros((48,), np.float32)
        b48[4 * v_idx : 4 * v_idx + 4] = bq_np[4 * h : 4 * h + 4]
        b48[32:36] = bk_np[c_lo : c_lo + 4]
        b48[36 + 4 * v_idx : 36 + 4 * v_idx + 4] = bv_np[c_lo : c_lo + 4]

        wo_t4 = np.ascontiguousarray(
            np.transpose(wo_np[:, 4 * h : 4 * h + 4], (1, 2, 3, 0))
        ).reshape(4, -1)
        wo13 = np.zeros((13, wo_t4.shape[1]), np.float32)
        wo13[1 + 4 * v_idx : 1 + 4 * v_idx + 4] = wo_t4

        in_maps.append({
            "xp": xp,
            "wqkv_t": taps(wqkv).reshape(CIN, -1),
            "b48": b48.reshape(48, 1),
            "wo_t": wo13,
            "id16": np.eye(16, dtype=np.float32),
            "id12": np.eye(12, dtype=np.float32),
            "ones13": np.ones((1, 13), np.float32),
        })
    return in_maps


def kernel(x, wq, bq, wk, bk, wv, bv, wo):
    from concourse.bass_utils import run_bass_kernel_spmd

    nc = _get_nc()
    in_maps = make_in_maps(x, wq, bq, wk, bk, wv, bv, wo)
    res = run_bass_kernel_spmd(nc, in_maps, list(range(8))).results
    out = np.zeros((COUT, H * W), np.float32)
    for m in res:
        out = out + m["out"]
    return out.reshape(1, COUT, H, W)


# revision 56
# speedup vs baseline: 1.9336x; 1.0033x over previous
"""Trainium2 Bass kernel for nn_MultiHeadAttention_75737453297867.

Sharding: one head per NeuronCore (8 heads / 8 cores). The reference's
aliased as_strided gather needs a per-core base offset 24576*h into the
flange-padded k/v storage; 24576*h mod 9216 is row-aligned (phi in
{0,48,96}), so three phi-shifted staging variants are built statically
and selection happens through host data alone: q and v convs run with
12 stacked channels (inactive variants host-zeroed), the QK contraction
runs over K=12, AV produces 13 rows (1 denominator + 12 variant-dph)
and the final conv weights (host-built) pick the active variant.

Staging path (cost model: DMA time = free bytes/partition * 0.386ns):
conv out [28,128,48] -> PE-transposed to row-major [128,16,48] ->
per-variant phi-shifted writes into flat DRAM ([144,64]-padded channel
images) -> one flat [12,9216] load back for uk (contiguous rows, no 2x
small-elem penalty) + direct strided DMAs building uvT [120,13,48].
QK reads uk chunks via strided APs (row-aliased windows), exp on ACT
from PSUM, AV accumulates [13,384] on PE, normalize writes straight
into the padded o image, final 3x3 conv, partial outputs summed on
host.
"""

import sys

import numpy as np

if "/opt/trn_rl_repo" not in sys.path:
    sys.path.insert(0, "/opt/trn_rl_repo")

import concourse.bass as bass
import concourse.tile as tile
from concourse import bacc
from concourse import mybir
from concourse.bass_types import AP

# Problem constants
CIN, COUT, H, W = 64, 64, 128, 48
DM, NH, DPH = 32, 8, 4
Q0, Q1, F0, F1 = 128, 24, 8, 8
M0, M1 = Q0 + 2 * F0, Q1 + 2 * F1          # 144, 40
KV = M0 * M1                               # 5760
CH = 144 * 64                              # 9216 flat padded-channel size
DST = 6144                                 # d-stride (Hp*Wp) in flat coords
KC = 128                                   # kv chunk: flat 128-slice of window
NKC = 45
RPB = 3                                    # kv chunks per psum round
NQC = 8                                    # q chunks of 384 (16 rows x 24)
QC = 384
QROWS = 16
PHIS = (0, 48, 96)
F32 = mybir.dt.float32
F32R = mybir.dt.float32r


def build_nc():
    nc = bacc.Bacc()

    xp_d = nc.dram_tensor("xp", [CIN, 130 * 50], F32R, kind="ExternalInput")
    wqkv_d = nc.dram_tensor("wqkv_t", [CIN, 9 * 48], F32R, kind="ExternalInput")
    b48_d = nc.dram_tensor("b48", [48, 1], F32, kind="ExternalInput")
    wo_d = nc.dram_tensor("wo_t", [13, 9 * 64], F32R, kind="ExternalInput")
    id16_d = nc.dram_tensor("id16", [16, 16], F32, kind="ExternalInput")
    id12_d = nc.dram_tensor("id12", [12, 12], F32R, kind="ExternalInput")
    o13_d = nc.dram_tensor("ones13", [1, 13], F32R, kind="ExternalInput")
    out_d = nc.dram_tensor("out", [COUT, H * W], F32, kind="ExternalOutput")

    from contextlib import ExitStack

    with tile.TileContext(nc) as tc, ExitStack() as ctx:
        P = ctx.enter_context(tc.tile_pool(name="persist", bufs=1))
        dram = ctx.enter_context(tc.tile_pool(name="dram", bufs=1, space="DRAM"))
        ctx1 = ctx.enter_context(ExitStack())
        P1 = ctx1.enter_context(tc.tile_pool(name="phase1", bufs=1))

        # ---- input loads (xp split across two DMA lanes) ----
        xp_sb = P1.tile([CIN, 130, 50], F32R, tag="xp")
        nc.sync.dma_start(
            out=xp_sb[:, 0:65, :],
            in_=xp_d[:, 0:3250].rearrange("p (a b) -> p a b", a=65),
        )
        nc.gpsimd.dma_start(
            out=xp_sb[:, 65:130, :],
            in_=xp_d[:, 3250:6500].rearrange("p (a b) -> p a b", a=65),
        )
        wqkv_sb = P.tile([CIN, 9, 48], F32R, tag="wqkv")
        nc.sync.dma_start(
            out=wqkv_sb, in_=wqkv_d[:, :].rearrange("p (t o) -> p t o", t=9)
        )
        b48 = P.tile([48, 1], F32, tag="b48")
        nc.sync.dma_start(out=b48, in_=b48_d[:, :])
        wo_sb = P.tile([13, 9, 64], F32R, tag="wo")
        nc.sync.dma_start(
            out=wo_sb, in_=wo_d[:, :].rearrange("p (t o) -> p t o", t=9)
        )
        id16 = P.tile([16, 16], F32, tag="id16")
        nc.sync.dma_start(out=id16, in_=id16_d[:, :])
        id12 = P.tile([12, 12], F32R, tag="id12")
        nc.sync.dma_start(out=id12, in_=id12_d[:, :])
        ones13 = P.tile([1, 13], F32R, tag="ones13")
        nc.sync.dma_start(out=ones13, in_=o13_d[:, :])

        zero_sb = P1.tile([128, 648], F32, tag="zeros")
        nc.vector.memset(zero_sb, 0.0)

        # ---- DRAM staging buffers (3 variants x 3 channels each) ----
        kp_all = dram.tile([9, CH], F32, tag="kp")
        vp_all = dram.tile([9, CH], F32, tag="vp")
        for buf in (kp_all, vp_all):
            dst = AP(tensor=buf.tensor, offset=buf.offset,
                     ap=[[648, 128], [1, 648]])
            nc.scalar.dma_start(out=dst, in_=zero_sb[:, :])

        # ---- stacked q/k/v conv: rows 0-11 = q, rows 32-47 = k4 + v12 ----
        # (kv starts at 32: engine PSUM access must be 32-partition aligned)
        q_sb = P.tile([12, 128, 48], F32R, tag="q_sb")
        kv_sb = P1.tile([16, 128, 48], F32, tag="kv_sb")
        with tc.tile_pool(name="psc", bufs=4, space="PSUM") as psc:
            for chv in range(16):
                ps = psc.tile([48, 8, 48], F32, tag="cps")
                for t in range(9):
                    dy, dx = t // 3, t % 3
                    rhs = xp_sb[:, 8 * chv + dy : 8 * chv + dy + 8, dx : dx + 48]
                    nc.tensor.matmul(
                        ps[:, :, :], wqkv_sb[:, t, 0:48], rhs,
                        start=(t == 0), stop=(t == 8),
                    )
                nc.vector.tensor_scalar_add(
                    q_sb[:, 8 * chv : 8 * chv + 8, :], ps[0:12, :, :],
                    b48[0:12, 0:1],
                )
                nc.vector.tensor_scalar_add(
                    kv_sb[:, 8 * chv : 8 * chv + 8, :], ps[32:48, :, :],
                    b48[32:48, 0:1],
                )

        # ---- transpose k/v to row-major [128 rows, 16 ch, 48 cols] ----
        kv_row = P1.tile([128, 16, 48], F32, tag="kv_row")
        with tc.tile_pool(name="pst", bufs=2, space="PSUM") as pst:
            for x in range(48):
                tp = pst.tile([128, 16], F32, tag="tp")
                nc.tensor.matmul(tp, kv_sb[:, :, x], id16[:, :],
                                 start=True, stop=True)
                nc.vector.tensor_copy(kv_row[:, :, x], tp)

        # ---- phi-shifted staging writes into the padded channel images ----
        engs = [nc.sync, nc.gpsimd, nc.scalar]
        ei = 0
        for buf_all, cbase in ((kp_all, lambda v: 0), (vp_all, lambda v: 4 + 4 * v)):
            for v, phi in enumerate(PHIS):
                cb = cbase(v)
                base = buf_all.offset + 3 * v * CH
                if phi == 0:
                    dst = AP(tensor=buf_all.tensor, offset=base + 8 * 64 + 8,
                             ap=[[64, 128], [CH, 3], [1, 48]])
                    engs[ei % 3].dma_start(out=dst, in_=kv_row[0:128, cb : cb + 3, :])
                    ei += 1
                else:
                    n1 = 136 - phi
                    dst1 = AP(tensor=buf_all.tensor, offset=base + 8,
                              ap=[[64, n1], [CH, 3], [1, 48]])
                    engs[ei % 3].dma_start(
                        out=dst1, in_=kv_row[phi - 8 : 128, cb : cb + 3, :])
                    ei += 1
                    n2 = phi - 8
                    dst2 = AP(tensor=buf_all.tensor,
                              offset=base + (152 - phi) * 64 + 8,
                              ap=[[64, n2], [CH, 3], [1, 48]])
                    engs[ei % 3].dma_start(
                        out=dst2, in_=kv_row[0 : phi - 8, cb + 1 : cb + 4, :])
                    ei += 1

        # ---- padded attention-output image; zero only the 1-px border ----
        o_pad = P.tile([13, 130, 50], F32R, tag="opad")
        zb = zero_sb[0:13, 0:1]
        for dst in (o_pad[:, 0, :], o_pad[:, 129, :],
                    o_pad[:, 1:129, 0], o_pad[:, 1:129, 49]):
            n = dst.free_size()
            src = AP(tensor=zb.tensor, offset=zb.offset, ap=[zb.ap[0], [0, n]])
            nc.vector.tensor_copy(dst, src)

        ctx1.close()  # free xp / kv_sb / kv_row / zeros SBUF
        ctx2 = ctx.enter_context(ExitStack())
        P2 = ctx2.enter_context(tc.tile_pool(name="phase2", bufs=1))
        uvp = ctx2.enter_context(tc.tile_pool(name="uvp", bufs=1))

        # ---- flat uk load: partition (v,d) = contiguous 9216-elem span ----
        uk_flat = P2.tile([12, CH], F32R, tag="uk")
        for i, eng in enumerate((nc.sync, nc.gpsimd, nc.scalar)):
            lo, hi = 3072 * i, 3072 * (i + 1)
            src = AP(tensor=kp_all.tensor, offset=kp_all.offset + lo,
                     ap=[[3 * CH, 3], [DST, 4], [1, hi - lo]])
            eng.dma_start(out=uk_flat[:, lo:hi], in_=src.bitcast(F32R))

        # ---- flat uv load (SP/Pool only; keep ACT free for ukr0 + exp) ----
        uv_flat = P2.tile([12, CH], F32R, tag="uv")
        for i, eng in enumerate((nc.sync, nc.gpsimd)):
            lo, hi = 4608 * i, 4608 * (i + 1)
            src = AP(tensor=vp_all.tensor, offset=vp_all.offset + lo,
                     ap=[[3 * CH, 3], [DST, 4], [1, hi - lo]])
            eng.dma_start(out=uv_flat[:, lo:hi], in_=src.bitcast(F32R))

        # ---- materialize flat window views (stationary matmul operands
        # must be 1-free-dim): engine copies with strided window reads;
        # uvT [120, 13, 48]: slot 0 = ones (denominator), slots 1-12 =
        # PE-transposed [12,120] kv chunks ----
        ukb, uvb = uk_flat[:, 0:1], uv_flat[:, 0:1]
        id12r = id12[:, :]
        uk_rep, uvT = [], []
        with tc.tile_pool(name="pstv", bufs=4, space="PSUM") as pstv:
            for j in range(2):
                ukr = P.tile([12, KV], F32R, tag=f"ukr{j}")
                src = AP(tensor=ukb.tensor, offset=ukb.offset + 24 * j,
                         ap=[ukb.ap[0], [48, M0], [1, M1]])
                if j == 0:
                    nc.scalar.copy(ukr[:, :], src)
                else:
                    nc.gpsimd.tensor_copy(ukr[:, :], src)
                uk_rep.append(ukr)

                uvr = uvp.tile([12, KV], F32R, tag="uvr")
                src = AP(tensor=uvb.tensor, offset=uvb.offset + 24 * j,
                         ap=[uvb.ap[0], [48, M0], [1, M1]])
                nc.vector.tensor_copy(uvr[:, :], src)

                t = P.tile([KC, 13, NKC], F32R, tag=f"uvt{j}")
                nc.vector.memset(t[:, 0, :].bitcast(F32), 1.0)
                uvT.append(t)
                for c in range(NKC):
                    tp = pstv.tile([KC, 12], F32R, tag="tpv")
                    nc.tensor.transpose(
                        tp, uvr[:, KC * c : KC * (c + 1)], id12r)
                    nc.vector.tensor_copy(uvT[j][:, 1:13, c], tp)

        ctx2.close()  # free uk_flat / uv_flat / uvr SBUF
        PL = ctx.enter_context(tc.tile_pool(name="late", bufs=1))

        # ---- attention (qc-outer / j-inner) with the final conv
        # interleaved: after q-chunk t, output rows for conv chunks
        # c <= 2t are complete, so the 3x3 conv streams behind it ----
        out_sb = PL.tile([COUT, 128, 48], F32, tag="outsb")
        dma_engs = (nc.sync, nc.gpsimd, nc.scalar, nc.sync)
        rec_a = PL.tile([32, QC], F32, tag="reca")
        nc.vector.memset(rec_a, 0.0)
        rec_b = PL.tile([32, QC], F32, tag="recb")
        with (
            tc.tile_pool(name="psqk", bufs=2, space="PSUM") as psqk,
            tc.tile_pool(name="psav", bufs=1, space="PSUM") as psav,
            tc.tile_pool(name="psf", bufs=1, space="PSUM") as psf,
            tc.tile_pool(name="expp", bufs=2) as expp,
            tc.tile_pool(name="redp", bufs=2) as redp,
        ):
            ci = 0  # next final-conv chunk to emit
            for qc in range(NQC):
                for j in range(2):
                    ps_av = psav.tile([13, QC], F32, tag="av")
                    for rnd in range(NKC // RPB):
                        ps_qk = psqk.tile([KC, RPB, 512], F32, tag="qk")
                        for b in range(RPB):
                            c = RPB * rnd + b
                            lhsT = uk_rep[j][:, KC * c : KC * (c + 1)]
                            rhs = q_sb[
                                :,
                                QROWS * qc : QROWS * (qc + 1),
                                24 * j : 24 * j + 24,
                            ]
                            out = ps_qk[0:KC, b, 0:QC].rearrange(
                                "p (a c) -> p a c", a=QROWS
                            )
                            nc.tensor.matmul(out, lhsT, rhs,
                                             start=True, stop=True)
                        ex = expp.tile([KC, RPB, QC], F32R, tag="ex")
                        nc.scalar.activation(
                            ex, ps_qk[0:KC, :, 0:QC],
                            mybir.ActivationFunctionType.Exp,
                        )
                        for b in range(RPB):
                            c = RPB * rnd + b
                            nc.tensor.matmul(
                                ps_av[:, :], uvT[j][:, :, c], ex[:, b, :],
                                start=(c == 0), stop=(c == NKC - 1),
                            )
                    # normalize: row 0 of ps_av is the softmax denominator;
                    # the per-q reciprocal broadcasts across partitions via
                    # a stride-0 AP on GPSIMD (software engine, SBUF only)
                    s0 = redp.tile([13, QC], F32, tag="s0")
                    nc.vector.tensor_copy(s0, ps_av[:, :])
                    nc.vector.reciprocal(rec_a[0:1, :], s0[0:1, :])
                    nc.vector.stream_shuffle(rec_b[:, :], rec_a[:, :],
                                             [0] * 32)
                    o_div = redp.tile([13, QC], F32, tag="odiv")
                    nc.vector.tensor_tensor(out=o_div, in0=s0[:, :],
                                            in1=rec_b[0:13, :],
                                            op=mybir.AluOpType.mult)
                    dst = o_pad[
                        :,
                        1 + QROWS * qc : 1 + QROWS * (qc + 1),
                        1 + 24 * j : 25 + 24 * j,
                    ]
                    nc.vector.tensor_copy(
                        dst, o_div.rearrange("p (a c) -> p a c", a=QROWS)
                    )
                # stream the final conv over completed o_pad rows
                while ci <= (15 if qc == NQC - 1 else 2 * qc):
                    ps = psf.tile([COUT, 8, 48], F32, tag="fps")
                    for t in range(9):
                        dy, dx = t // 3, t % 3
                        rhs = o_pad[:, 8 * ci + dy : 8 * ci + dy + 8,
                                    dx : dx + 48]
                        nc.tensor.matmul(ps[:, :, :], wo_sb[:, t, :], rhs,
                                         start=(t == 0), stop=(t == 8))
                    nc.vector.tensor_copy(
                        out_sb[:, 8 * ci : 8 * ci + 8, :], ps)
                    if ci % 4 == 3:  # stream the finished quarter out
                        q4 = ci // 4
                        dma_engs[q4].dma_start(
                            out=out_d[:, 1536 * q4 : 1536 * (q4 + 1)],
                            in_=out_sb[:, 32 * q4 : 32 * (q4 + 1), :]
                            .rearrange("p a b -> p (a b)"),
                        )
                    ci += 1

    nc.compile()
    return nc


_NC = None


def _get_nc():
    global _NC
    if _NC is None:
        _NC = build_nc()
    return _NC


def make_in_maps(x, wq, bq, wk, bk, wv, bv, wo):
    x = np.asarray(x, np.float32)[0]           # [64, 128, 48]
    xp = np.zeros((CIN, 130, 50), np.float32)
    xp[:, 1:129, 1:49] = x
    xp = xp.reshape(CIN, -1)
    s = np.float32(DPH ** -0.5)

    def taps(w):                                # [O, I, 3, 3] -> [I, 9, O]
        return np.ascontiguousarray(np.transpose(w, (1, 2, 3, 0)).reshape(
            w.shape[1], 9, w.shape[0]))

    wq_np = np.asarray(wq, np.float32)
    wk_np = np.asarray(wk, np.float32) * s
    wv_np = np.asarray(wv, np.float32)
    wo_np = np.asarray(wo, np.float32)
    bq_np = np.asarray(bq, np.float32)
    bk_np = np.asarray(bk, np.float32) * s
    bv_np = np.asarray(bv, np.float32)

    in_maps = []
    for h in range(8):
        c_lo = (24576 * h) // 9216
        phi = (24576 * h - 9216 * c_lo) // 64
        v_idx = PHIS.index(phi)

        wqkv = np.zeros((48, CIN, 3, 3), np.float32)
        wqkv[4 * v_idx : 4 * v_idx + 4] = wq_np[4 * h : 4 * h + 4]
        wqkv[32:36] = wk_np[c_lo : c_lo + 4]
        wqkv[36 + 4 * v_idx : 36 + 4 * v_idx + 4] = wv_np[c_lo : c_lo + 4]

        b48 = np.zeros((48,), np.float32)
        b48[4 * v_idx : 4 * v_idx + 4] = bq_np[4 * h : 4 * h + 4]
        b48[32:36] = bk_np[c_lo : c_lo + 4]
        b48[36 + 4 * v_idx : 36 + 4 * v_idx + 4] = bv_np[c_lo : c_lo + 4]

        wo_t4 = np.ascontiguousarray(
            np.transpose(wo_np[:, 4 * h : 4 * h + 4], (1, 2, 3, 0))
        ).reshape(4, -1)
        wo13 = np.zeros((13, wo_t4.shape[1]), np.float32)
        wo13[1 + 4 * v_idx : 1 + 4 * v_idx + 4] = wo_t4

        in_maps.append({
            "xp": xp,
            "wqkv_t": taps(wqkv).reshape(CIN, -1),
            "b48": b48.reshape(48, 1),
            "wo_t": wo13,
            "id16": np.eye(16, dtype=np.float32),
            "id12": np.eye(12, dtype=np.float32),
            "ones13": np.ones((1, 13), np.float32),
        })
    return in_maps


def kernel(x, wq, bq, wk, bk, wv, bv, wo):
    from concourse.bass_utils import run_bass_kernel_spmd

    nc = _get_nc()
    in_maps = make_in_maps(x, wq, bq, wk, bk, wv, bv, wo)
    res = run_bass_kernel_spmd(nc, in_maps, list(range(8))).results
    out = np.zeros((COUT, H * W), np.float32)
    for m in res:
        out = out + m["out"]
    return out.reshape(1, COUT, H, W)


# revision 61
# speedup vs baseline: 1.9355x; 1.0010x over previous
"""Trainium2 Bass kernel for nn_MultiHeadAttention_75737453297867.

Sharding: one head per NeuronCore (8 heads / 8 cores). The reference's
aliased as_strided gather needs a per-core base offset 24576*h into the
flange-padded k/v storage; 24576*h mod 9216 is row-aligned (phi in
{0,48,96}), so three phi-shifted staging variants are built statically
and selection happens through host data alone: q and v convs run with
12 stacked channels (inactive variants host-zeroed), the QK contraction
runs over K=12, AV produces 13 rows (1 denominator + 12 variant-dph)
and the final conv weights (host-built) pick the active variant.

Staging path (cost model: DMA time = free bytes/partition * 0.386ns):
conv out [28,128,48] -> PE-transposed to row-major [128,16,48] ->
per-variant phi-shifted writes into flat DRAM ([144,64]-padded channel
images) -> one flat [12,9216] load back for uk (contiguous rows, no 2x
small-elem penalty) + direct strided DMAs building uvT [120,13,48].
QK reads uk chunks via strided APs (row-aliased windows), exp on ACT
from PSUM, AV accumulates [13,384] on PE, normalize writes straight
into the padded o image, final 3x3 conv, partial outputs summed on
host.
"""

import sys

import numpy as np

if "/opt/trn_rl_repo" not in sys.path:
    sys.path.insert(0, "/opt/trn_rl_repo")

import concourse.bass as bass
import concourse.tile as tile
from concourse import bacc
from concourse import mybir
from concourse.bass_types import AP

# Problem constants
CIN, COUT, H, W = 64, 64, 128, 48
DM, NH, DPH = 32, 8, 4
Q0, Q1, F0, F1 = 128, 24, 8, 8
M0, M1 = Q0 + 2 * F0, Q1 + 2 * F1          # 144, 40
KV = M0 * M1                               # 5760
CH = 144 * 64                              # 9216 flat padded-channel size
DST = 6144                                 # d-stride (Hp*Wp) in flat coords
KC = 128                                   # kv chunk: flat 128-slice of window
NKC = 45
RPB = 3                                    # kv chunks per psum round
NQC = 8                                    # q chunks of 384 (16 rows x 24)
QC = 384
QROWS = 16
PHIS = (0, 48, 96)
F32 = mybir.dt.float32
F32R = mybir.dt.float32r


def build_nc():
    nc = bacc.Bacc()

    xp_d = nc.dram_tensor("xp", [CIN, 130 * 50], F32R, kind="ExternalInput")
    wqkv_d = nc.dram_tensor("wqkv_t", [CIN, 9 * 48], F32R, kind="ExternalInput")
    b48_d = nc.dram_tensor("b48", [48, 1], F32, kind="ExternalInput")
    wo_d = nc.dram_tensor("wo_t", [13, 9 * 64], F32R, kind="ExternalInput")
    id16_d = nc.dram_tensor("id16", [16, 16], F32, kind="ExternalInput")
    id12_d = nc.dram_tensor("id12", [12, 12], F32R, kind="ExternalInput")
    o13_d = nc.dram_tensor("ones13", [1, 13], F32R, kind="ExternalInput")
    out_d = nc.dram_tensor("out", [COUT, H * W], F32, kind="ExternalOutput")

    from contextlib import ExitStack

    with tile.TileContext(nc) as tc, ExitStack() as ctx:
        P = ctx.enter_context(tc.tile_pool(name="persist", bufs=1))
        dram = ctx.enter_context(tc.tile_pool(name="dram", bufs=1, space="DRAM"))
        ctx1 = ctx.enter_context(ExitStack())
        P1 = ctx1.enter_context(tc.tile_pool(name="phase1", bufs=1))

        # ---- input loads (xp split across two DMA lanes) ----
        xp_sb = P1.tile([CIN, 130, 50], F32R, tag="xp")
        nc.sync.dma_start(
            out=xp_sb[:, 0:65, :],
            in_=xp_d[:, 0:3250].rearrange("p (a b) -> p a b", a=65),
        )
        nc.gpsimd.dma_start(
            out=xp_sb[:, 65:130, :],
            in_=xp_d[:, 3250:6500].rearrange("p (a b) -> p a b", a=65),
        )
        wqkv_sb = P.tile([CIN, 9, 48], F32R, tag="wqkv")
        nc.sync.dma_start(
            out=wqkv_sb, in_=wqkv_d[:, :].rearrange("p (t o) -> p t o", t=9)
        )
        b48 = P.tile([48, 1], F32, tag="b48")
        nc.sync.dma_start(out=b48, in_=b48_d[:, :])
        wo_sb = P.tile([13, 9, 64], F32R, tag="wo")
        nc.sync.dma_start(
            out=wo_sb, in_=wo_d[:, :].rearrange("p (t o) -> p t o", t=9)
        )
        id16 = P.tile([16, 16], F32, tag="id16")
        nc.sync.dma_start(out=id16, in_=id16_d[:, :])
        id12 = P.tile([12, 12], F32R, tag="id12")
        nc.sync.dma_start(out=id12, in_=id12_d[:, :])
        ones13 = P.tile([1, 13], F32R, tag="ones13")
        nc.sync.dma_start(out=ones13, in_=o13_d[:, :])

        zero_sb = P1.tile([128, 648], F32, tag="zeros")
        nc.vector.memset(zero_sb, 0.0)

        # ---- DRAM staging buffers (3 variants x 3 channels each) ----
        kp_all = dram.tile([9, CH], F32, tag="kp")
        vp_all = dram.tile([9, CH], F32, tag="vp")
        for buf in (kp_all, vp_all):
            dst = AP(tensor=buf.tensor, offset=buf.offset,
                     ap=[[648, 128], [1, 648]])
            nc.scalar.dma_start(out=dst, in_=zero_sb[:, :])

        # ---- stacked q/k/v conv: rows 0-11 = q, rows 32-47 = k4 + v12 ----
        # (kv starts at 32: engine PSUM access must be 32-partition aligned)
        q_sb = P.tile([12, 128, 48], F32R, tag="q_sb")
        kv_sb = P1.tile([16, 128, 48], F32, tag="kv_sb")
        with tc.tile_pool(name="psc", bufs=4, space="PSUM") as psc:
            for chv in range(16):
                ps = psc.tile([48, 8, 48], F32, tag="cps")
                for t in range(9):
                    dy, dx = t // 3, t % 3
                    rhs = xp_sb[:, 8 * chv + dy : 8 * chv + dy + 8, dx : dx + 48]
                    nc.tensor.matmul(
                        ps[:, :, :], wqkv_sb[:, t, 0:48], rhs,
                        start=(t == 0), stop=(t == 8),
                    )
                nc.vector.tensor_scalar_add(
                    q_sb[:, 8 * chv : 8 * chv + 8, :], ps[0:12, :, :],
                    b48[0:12, 0:1],
                )
                nc.vector.tensor_scalar_add(
                    kv_sb[:, 8 * chv : 8 * chv + 8, :], ps[32:48, :, :],
                    b48[32:48, 0:1],
                )

        # ---- transpose k/v to row-major [128 rows, 16 ch, 48 cols] ----
        kv_row = P1.tile([128, 16, 48], F32, tag="kv_row")
        with tc.tile_pool(name="pst", bufs=2, space="PSUM") as pst:
            for x in range(48):
                tp = pst.tile([128, 16], F32, tag="tp")
                nc.tensor.matmul(tp, kv_sb[:, :, x], id16[:, :],
                                 start=True, stop=True)
                nc.vector.tensor_copy(kv_row[:, :, x], tp)

        # ---- phi-shifted staging writes into the padded channel images ----
        engs = [nc.sync, nc.gpsimd, nc.scalar]
        ei = 0
        for buf_all, cbase in ((kp_all, lambda v: 0), (vp_all, lambda v: 4 + 4 * v)):
            for v, phi in enumerate(PHIS):
                cb = cbase(v)
                base = buf_all.offset + 3 * v * CH
                if phi == 0:
                    dst = AP(tensor=buf_all.tensor, offset=base + 8 * 64 + 8,
                             ap=[[64, 128], [CH, 3], [1, 48]])
                    engs[ei % 3].dma_start(out=dst, in_=kv_row[0:128, cb : cb + 3, :])
                    ei += 1
                else:
                    n1 = 136 - phi
                    dst1 = AP(tensor=buf_all.tensor, offset=base + 8,
                              ap=[[64, n1], [CH, 3], [1, 48]])
                    engs[ei % 3].dma_start(
                        out=dst1, in_=kv_row[phi - 8 : 128, cb : cb + 3, :])
                    ei += 1
                    n2 = phi - 8
                    dst2 = AP(tensor=buf_all.tensor,
                              offset=base + (152 - phi) * 64 + 8,
                              ap=[[64, n2], [CH, 3], [1, 48]])
                    engs[ei % 3].dma_start(
                        out=dst2, in_=kv_row[0 : phi - 8, cb + 1 : cb + 4, :])
                    ei += 1

        # ---- padded attention-output image; zero only the 1-px border ----
        o_pad = P.tile([13, 130, 50], F32R, tag="opad")
        zb = zero_sb[0:13, 0:1]
        for dst in (o_pad[:, 0, :], o_pad[:, 129, :],
                    o_pad[:, 1:129, 0], o_pad[:, 1:129, 49]):
            n = dst.free_size()
            src = AP(tensor=zb.tensor, offset=zb.offset, ap=[zb.ap[0], [0, n]])
            nc.vector.tensor_copy(dst, src)

        ctx1.close()  # free xp / kv_sb / kv_row / zeros SBUF
        ctx2 = ctx.enter_context(ExitStack())
        P2 = ctx2.enter_context(tc.tile_pool(name="phase2", bufs=1))
        uvp = ctx2.enter_context(tc.tile_pool(name="uvp", bufs=1))

        # ---- flat uk load: partition (v,d) = contiguous 9216-elem span;
        # part boundaries align with window row-groups 0-63 / 64-127 so the
        # window copies below can start as soon as their parts land ----
        uk_flat = P2.tile([12, CH], F32R, tag="uk")
        uk_cuts = (0, 3112, 6208, CH)
        for i, eng in enumerate((nc.sync, nc.gpsimd, nc.scalar)):
            lo, hi = uk_cuts[i], uk_cuts[i + 1]
            src = AP(tensor=kp_all.tensor, offset=kp_all.offset + lo,
                     ap=[[3 * CH, 3], [DST, 4], [1, hi - lo]])
            eng.dma_start(out=uk_flat[:, lo:hi], in_=src.bitcast(F32R))

        # ---- flat uv load (SP/Pool only; keep ACT free for ukr0 + exp) ----
        uv_flat = P2.tile([12, CH], F32R, tag="uv")
        for i, eng in enumerate((nc.sync, nc.gpsimd)):
            lo, hi = 4608 * i, 4608 * (i + 1)
            src = AP(tensor=vp_all.tensor, offset=vp_all.offset + lo,
                     ap=[[3 * CH, 3], [DST, 4], [1, hi - lo]])
            eng.dma_start(out=uv_flat[:, lo:hi], in_=src.bitcast(F32R))

        # ---- materialize flat window views (stationary matmul operands
        # must be 1-free-dim): engine copies with strided window reads;
        # uvT [120, 13, 48]: slot 0 = ones (denominator), slots 1-12 =
        # PE-transposed [12,120] kv chunks ----
        ukb, uvb = uk_flat[:, 0:1], uv_flat[:, 0:1]
        id12r = id12[:, :]
        uk_rep, uvT = [], []
        with tc.tile_pool(name="pstv", bufs=4, space="PSUM") as pstv:
            row_groups = ((0, 64), (64, 128), (128, 144))
            for j in range(2):
                ukr = P.tile([12, KV], F32R, tag=f"ukr{j}")
                if j == 0:
                    # 3 row-group copies pipeline against the uk DMA parts
                    for r0, r1 in row_groups:
                        src = AP(tensor=ukb.tensor,
                                 offset=ukb.offset + 24 * j + 48 * r0,
                                 ap=[ukb.ap[0], [48, r1 - r0], [1, M1]])
                        nc.scalar.copy(ukr[:, 40 * r0 : 40 * r1], src)
                else:
                    src = AP(tensor=ukb.tensor, offset=ukb.offset + 24 * j,
                             ap=[ukb.ap[0], [48, M0], [1, M1]])
                    nc.gpsimd.tensor_copy(ukr[:, :], src)
                uk_rep.append(ukr)

                uvr = uvp.tile([12, KV], F32R, tag="uvr")
                for r0, r1 in row_groups:
                    src = AP(tensor=uvb.tensor,
                             offset=uvb.offset + 24 * j + 48 * r0,
                             ap=[uvb.ap[0], [48, r1 - r0], [1, M1]])
                    nc.vector.tensor_copy(uvr[:, 40 * r0 : 40 * r1], src)

                t = P.tile([KC, 13, NKC], F32R, tag=f"uvt{j}")
                nc.vector.memset(t[:, 0, :].bitcast(F32), 1.0)
                uvT.append(t)
                for c in range(NKC):
                    tp = pstv.tile([KC, 12], F32R, tag="tpv")
                    nc.tensor.transpose(
                        tp, uvr[:, KC * c : KC * (c + 1)], id12r)
                    nc.vector.tensor_copy(uvT[j][:, 1:13, c], tp)

        ctx2.close()  # free uk_flat / uv_flat / uvr SBUF
        PL = ctx.enter_context(tc.tile_pool(name="late", bufs=1))

        # ---- attention (qc-outer / j-inner) with the final conv
        # interleaved: after q-chunk t, output rows for conv chunks
        # c <= 2t are complete, so the 3x3 conv streams behind it ----
        out_sb = PL.tile([COUT, 128, 48], F32, tag="outsb")
        dma_engs = (nc.sync, nc.gpsimd, nc.scalar, nc.sync)
        rec_a = PL.tile([32, QC], F32, tag="reca")
        nc.vector.memset(rec_a, 0.0)
        rec_b = PL.tile([32, QC], F32, tag="recb")
        with (
            tc.tile_pool(name="psqk", bufs=2, space="PSUM") as psqk,
            tc.tile_pool(name="psav", bufs=1, space="PSUM") as psav,
            tc.tile_pool(name="psf", bufs=1, space="PSUM") as psf,
            tc.tile_pool(name="expp", bufs=2) as expp,
            tc.tile_pool(name="redp", bufs=2) as redp,
        ):
            ci = 0  # next final-conv chunk to emit (one-qc lag: deps stale)
            for qc in range(NQC):
                while ci <= 2 * (qc - 1):
                    ps = psf.tile([COUT, 8, 48], F32, tag="fps")
                    for t in range(9):
                        dy, dx = t // 3, t % 3
                        rhs = o_pad[:, 8 * ci + dy : 8 * ci + dy + 8,
                                    dx : dx + 48]
                        nc.tensor.matmul(ps[:, :, :], wo_sb[:, t, :], rhs,
                                         start=(t == 0), stop=(t == 8))
                    nc.vector.tensor_copy(
                        out_sb[:, 8 * ci : 8 * ci + 8, :], ps)
                    if ci % 4 == 3:
                        q4 = ci // 4
                        dma_engs[q4].dma_start(
                            out=out_d[:, 1536 * q4 : 1536 * (q4 + 1)],
                            in_=out_sb[:, 32 * q4 : 32 * (q4 + 1), :]
                            .rearrange("p a b -> p (a b)"),
                        )
                    ci += 1
                for j in range(2):
                    ps_av = psav.tile([13, QC], F32, tag="av")
                    for rnd in range(NKC // RPB):
                        ps_qk = psqk.tile([KC, RPB, 512], F32, tag="qk")
                        for b in range(RPB):
                            c = RPB * rnd + b
                            lhsT = uk_rep[j][:, KC * c : KC * (c + 1)]
                            rhs = q_sb[
                                :,
                                QROWS * qc : QROWS * (qc + 1),
                                24 * j : 24 * j + 24,
                            ]
                            out = ps_qk[0:KC, b, 0:QC].rearrange(
                                "p (a c) -> p a c", a=QROWS
                            )
                            nc.tensor.matmul(out, lhsT, rhs,
                                             start=True, stop=True)
                        ex = expp.tile([KC, RPB, QC], F32R, tag="ex")
                        nc.scalar.activation(
                            ex, ps_qk[0:KC, :, 0:QC],
                            mybir.ActivationFunctionType.Exp,
                        )
                        for b in range(RPB):
                            c = RPB * rnd + b
                            nc.tensor.matmul(
                                ps_av[:, :], uvT[j][:, :, c], ex[:, b, :],
                                start=(c == 0), stop=(c == NKC - 1),
                            )
                    # normalize: row 0 of ps_av is the softmax denominator;
                    # the per-q reciprocal broadcasts across partitions via
                    # a stride-0 AP on GPSIMD (software engine, SBUF only)
                    s0 = redp.tile([13, QC], F32, tag="s0")
                    nc.vector.tensor_copy(s0, ps_av[:, :])
                    nc.vector.reciprocal(rec_a[0:1, :], s0[0:1, :])
                    nc.vector.stream_shuffle(rec_b[:, :], rec_a[:, :],
                                             [0] * 32)
                    o_div = redp.tile([13, QC], F32, tag="odiv")
                    nc.vector.tensor_tensor(out=o_div, in0=s0[:, :],
                                            in1=rec_b[0:13, :],
                                            op=mybir.AluOpType.mult)
                    dst = o_pad[
                        :,
                        1 + QROWS * qc : 1 + QROWS * (qc + 1),
                        1 + 24 * j : 25 + 24 * j,
                    ]
                    nc.vector.tensor_copy(
                        dst, o_div.rearrange("p (a c) -> p a c", a=QROWS)
                    )
            # drain remaining final-conv chunks
            while ci <= 15:
                ps = psf.tile([COUT, 8, 48], F32, tag="fps")
                for t in range(9):
                    dy, dx = t // 3, t % 3
                    rhs = o_pad[:, 8 * ci + dy : 8 * ci + dy + 8,
                                dx : dx + 48]
                    nc.tensor.matmul(ps[:, :, :], wo_sb[:, t, :], rhs,
                                     start=(t == 0), stop=(t == 8))
                nc.vector.tensor_copy(out_sb[:, 8 * ci : 8 * ci + 8, :], ps)
                if ci % 4 == 3:  # stream the finished quarter out
                    q4 = ci // 4
                    dma_engs[q4].dma_start(
                        out=out_d[:, 1536 * q4 : 1536 * (q4 + 1)],
                        in_=out_sb[:, 32 * q4 : 32 * (q4 + 1), :]
                        .rearrange("p a b -> p (a b)"),
                    )
                ci += 1

    nc.compile()
    return nc


_NC = None


def _get_nc():
    global _NC
    if _NC is None:
        _NC = build_nc()
    return _NC


def make_in_maps(x, wq, bq, wk, bk, wv, bv, wo):
    x = np.asarray(x, np.float32)[0]           # [64, 128, 48]
    xp = np.zeros((CIN, 130, 50), np.float32)
    xp[:, 1:129, 1:49] = x
    xp = xp.reshape(CIN, -1)
    s = np.float32(DPH ** -0.5)

    def taps(w):                                # [O, I, 3, 3] -> [I, 9, O]
        return np.ascontiguousarray(np.transpose(w, (1, 2, 3, 0)).reshape(
            w.shape[1], 9, w.shape[0]))

    wq_np = np.asarray(wq, np.float32)
    wk_np = np.asarray(wk, np.float32) * s
    wv_np = np.asarray(wv, np.float32)
    wo_np = np.asarray(wo, np.float32)
    bq_np = np.asarray(bq, np.float32)
    bk_np = np.asarray(bk, np.float32) * s
    bv_np = np.asarray(bv, np.float32)

    in_maps = []
    for h in range(8):
        c_lo = (24576 * h) // 9216
        phi = (24576 * h - 9216 * c_lo) // 64
        v_idx = PHIS.index(phi)

        wqkv = np.zeros((48, CIN, 3, 3), np.float32)
        wqkv[4 * v_idx : 4 * v_idx + 4] = wq_np[4 * h : 4 * h + 4]
        wqkv[32:36] = wk_np[c_lo : c_lo + 4]
        wqkv[36 + 4 * v_idx : 36 + 4 * v_idx + 4] = wv_np[c_lo : c_lo + 4]

        b48 = np.zeros((48,), np.float32)
        b48[4 * v_idx : 4 * v_idx + 4] = bq_np[4 * h : 4 * h + 4]
        b48[32:36] = bk_np[c_lo : c_lo + 4]
        b48[36 + 4 * v_idx : 36 + 4 * v_idx + 4] = bv_np[c_lo : c_lo + 4]

        wo_t4 = np.ascontiguousarray(
            np.transpose(wo_np[:, 4 * h : 4 * h + 4], (1, 2, 3, 0))
        ).reshape(4, -1)
        wo13 = np.zeros((13, wo_t4.shape[1]), np.float32)
        wo13[1 + 4 * v_idx : 1 + 4 * v_idx + 4] = wo_t4

        in_maps.append({
            "xp": xp,
            "wqkv_t": taps(wqkv).reshape(CIN, -1),
            "b48": b48.reshape(48, 1),
            "wo_t": wo13,
            "id16": np.eye(16, dtype=np.float32),
            "id12": np.eye(12, dtype=np.float32),
            "ones13": np.ones((1, 13), np.float32),
        })
    return in_maps


def kernel(x, wq, bq, wk, bk, wv, bv, wo):
    from concourse.bass_utils import run_bass_kernel_spmd

    nc = _get_nc()
    in_maps = make_in_maps(x, wq, bq, wk, bk, wv, bv, wo)
    res = run_bass_kernel_spmd(nc, in_maps, list(range(8))).results
    out = np.zeros((COUT, H * W), np.float32)
    for m in res:
        out = out + m["out"]
    return out.reshape(1, COUT, H, W)


# revision 67
# speedup vs baseline: 1.9893x; 1.0278x over previous
"""Trainium2 Bass kernel for nn_MultiHeadAttention_75737453297867.

Sharding: one head per NeuronCore (8 heads / 8 cores). The reference's
aliased as_strided gather needs a per-core base offset 24576*h into the
flange-padded k/v storage; 24576*h mod 9216 is row-aligned (phi in
{0,48,96}), so three phi-shifted staging variants are built statically
and selection happens through host data alone: q and v convs run with
12 stacked channels (inactive variants host-zeroed), the QK contraction
runs over K=12, AV produces 13 rows (1 denominator + 12 variant-dph)
and the final conv weights (host-built) pick the active variant.

Staging path (cost model: DMA time = free bytes/partition * 0.386ns):
conv out [28,128,48] -> PE-transposed to row-major [128,16,48] ->
per-variant phi-shifted writes into flat DRAM ([144,64]-padded channel
images) -> one flat [12,9216] load back for uk (contiguous rows, no 2x
small-elem penalty) + direct strided DMAs building uvT [120,13,48].
QK reads uk chunks via strided APs (row-aliased windows), exp on ACT
from PSUM, AV accumulates [13,384] on PE, normalize writes straight
into the padded o image, final 3x3 conv, partial outputs summed on
host.
"""

import sys

import numpy as np

if "/opt/trn_rl_repo" not in sys.path:
    sys.path.insert(0, "/opt/trn_rl_repo")

import concourse.bass as bass
import concourse.tile as tile
from concourse import bacc
from concourse import mybir
from concourse.bass_types import AP

# Problem constants
CIN, COUT, H, W = 64, 64, 128, 48
DM, NH, DPH = 32, 8, 4
Q0, Q1, F0, F1 = 128, 24, 8, 8
M0, M1 = Q0 + 2 * F0, Q1 + 2 * F1          # 144, 40
KV = M0 * M1                               # 5760
CH = 144 * 64                              # 9216 flat padded-channel size
DST = 6144                                 # d-stride (Hp*Wp) in flat coords
KC = 128                                   # kv chunk: flat 128-slice of window
NKC = 45
RPB = 3                                    # kv chunks per psum round
NQC = 8                                    # q chunks of 384 (16 rows x 24)
QC = 384
QROWS = 16
PHIS = (0, 48, 96)
F32 = mybir.dt.float32
F32R = mybir.dt.float32r


def build_nc():
    nc = bacc.Bacc()

    xp_d = nc.dram_tensor("xp", [CIN, 130 * 50], F32R, kind="ExternalInput")
    wqkv_d = nc.dram_tensor("wqkv_t", [CIN, 9 * 48], F32R, kind="ExternalInput")
    b48_d = nc.dram_tensor("b48", [48, 1], F32, kind="ExternalInput")
    wo_d = nc.dram_tensor("wo_t", [13, 9 * 64], F32R, kind="ExternalInput")
    id16_d = nc.dram_tensor("id16", [16, 16], F32, kind="ExternalInput")
    id12_d = nc.dram_tensor("id12", [12, 12], F32R, kind="ExternalInput")
    o13_d = nc.dram_tensor("ones13", [1, 13], F32R, kind="ExternalInput")
    out_d = nc.dram_tensor("out", [COUT, H * W], F32, kind="ExternalOutput")

    from contextlib import ExitStack

    with tile.TileContext(nc) as tc, ExitStack() as ctx:
        P = ctx.enter_context(tc.tile_pool(name="persist", bufs=1))
        dram = ctx.enter_context(tc.tile_pool(name="dram", bufs=1, space="DRAM"))
        ctx1 = ctx.enter_context(ExitStack())
        P1 = ctx1.enter_context(tc.tile_pool(name="phase1", bufs=1))

        # ---- input loads (xp split across two DMA lanes) ----
        xp_sb = P1.tile([CIN, 130, 50], F32R, tag="xp")
        nc.sync.dma_start(
            out=xp_sb[:, 0:65, :],
            in_=xp_d[:, 0:3250].rearrange("p (a b) -> p a b", a=65),
        )
        nc.gpsimd.dma_start(
            out=xp_sb[:, 65:130, :],
            in_=xp_d[:, 3250:6500].rearrange("p (a b) -> p a b", a=65),
        )
        wqkv_sb = P.tile([CIN, 9, 48], F32R, tag="wqkv")
        nc.sync.dma_start(
            out=wqkv_sb, in_=wqkv_d[:, :].rearrange("p (t o) -> p t o", t=9)
        )
        b48 = P.tile([48, 1], F32, tag="b48")
        nc.sync.dma_start(out=b48, in_=b48_d[:, :])
        wo_sb = P.tile([13, 9, 64], F32R, tag="wo")
        nc.sync.dma_start(
            out=wo_sb, in_=wo_d[:, :].rearrange("p (t o) -> p t o", t=9)
        )
        id16 = P.tile([16, 16], F32, tag="id16")
        nc.sync.dma_start(out=id16, in_=id16_d[:, :])
        id12 = P.tile([12, 12], F32R, tag="id12")
        nc.sync.dma_start(out=id12, in_=id12_d[:, :])
        ones13 = P.tile([1, 13], F32R, tag="ones13")
        nc.sync.dma_start(out=ones13, in_=o13_d[:, :])

        zero_sb = P1.tile([128, 648], F32, tag="zeros")
        nc.vector.memset(zero_sb, 0.0)

        # ---- DRAM staging buffers (3 variants x 3 channels each) ----
        kp_all = dram.tile([9, CH], F32, tag="kp")
        vp_all = dram.tile([9, CH], F32, tag="vp")
        for buf in (kp_all, vp_all):
            dst = AP(tensor=buf.tensor, offset=buf.offset,
                     ap=[[648, 128], [1, 648]])
            nc.scalar.dma_start(out=dst, in_=zero_sb[:, :])

        # ---- stacked q/k/v conv: rows 0-11 = q, rows 32-47 = k4 + v12 ----
        # (kv starts at 32: engine PSUM access must be 32-partition aligned)
        q_sb = P.tile([12, 128, 48], F32R, tag="q_sb")
        kv_sb = P1.tile([16, 128, 48], F32, tag="kv_sb")
        with tc.tile_pool(name="psc", bufs=4, space="PSUM") as psc:
            for chv in range(16):
                ps = psc.tile([48, 8, 48], F32, tag="cps")
                for t in range(9):
                    dy, dx = t // 3, t % 3
                    rhs = xp_sb[:, 8 * chv + dy : 8 * chv + dy + 8, dx : dx + 48]
                    nc.tensor.matmul(
                        ps[:, :, :], wqkv_sb[:, t, 0:48], rhs,
                        start=(t == 0), stop=(t == 8),
                    )
                nc.vector.tensor_scalar_add(
                    q_sb[:, 8 * chv : 8 * chv + 8, :], ps[0:12, :, :],
                    b48[0:12, 0:1],
                )
                nc.scalar.add(
                    kv_sb[:, 8 * chv : 8 * chv + 8, :], ps[32:48, :, :],
                    b48[32:48, 0:1],
                )

        # ---- transpose k/v to row-major [128 rows, 16 ch, 48 cols] ----
        kv_row = P1.tile([128, 16, 48], F32, tag="kv_row")
        with tc.tile_pool(name="pst", bufs=2, space="PSUM") as pst:
            for x in range(48):
                tp = pst.tile([128, 16], F32, tag="tp")
                nc.tensor.matmul(tp, kv_sb[:, :, x], id16[:, :],
                                 start=True, stop=True)
                nc.vector.tensor_copy(kv_row[:, :, x], tp)

        # ---- phi-shifted staging writes into the padded channel images ----
        engs = [nc.sync, nc.gpsimd, nc.scalar]
        ei = 0
        for buf_all, cbase in ((kp_all, lambda v: 0), (vp_all, lambda v: 4 + 4 * v)):
            for v, phi in enumerate(PHIS):
                cb = cbase(v)
                base = buf_all.offset + 3 * v * CH
                if phi == 0:
                    dst = AP(tensor=buf_all.tensor, offset=base + 8 * 64 + 8,
                             ap=[[64, 128], [CH, 3], [1, 48]])
                    engs[ei % 3].dma_start(out=dst, in_=kv_row[0:128, cb : cb + 3, :])
                    ei += 1
                else:
                    n1 = 136 - phi
                    dst1 = AP(tensor=buf_all.tensor, offset=base + 8,
                              ap=[[64, n1], [CH, 3], [1, 48]])
                    engs[ei % 3].dma_start(
                        out=dst1, in_=kv_row[phi - 8 : 128, cb : cb + 3, :])
                    ei += 1
                    n2 = phi - 8
                    dst2 = AP(tensor=buf_all.tensor,
                              offset=base + (152 - phi) * 64 + 8,
                              ap=[[64, n2], [CH, 3], [1, 48]])
                    engs[ei % 3].dma_start(
                        out=dst2, in_=kv_row[0 : phi - 8, cb + 1 : cb + 4, :])
                    ei += 1

        # ---- padded attention-output image; zero only the 1-px border ----
        o_pad = P.tile([13, 130, 50], F32R, tag="opad")
        zb = zero_sb[0:13, 0:1]
        for dst in (o_pad[:, 0, :], o_pad[:, 129, :],
                    o_pad[:, 1:129, 0], o_pad[:, 1:129, 49]):
            n = dst.free_size()
            src = AP(tensor=zb.tensor, offset=zb.offset, ap=[zb.ap[0], [0, n]])
            nc.vector.tensor_copy(dst, src)

        ctx1.close()  # free xp / kv_sb / kv_row / zeros SBUF
        ctx2 = ctx.enter_context(ExitStack())
        P2 = ctx2.enter_context(tc.tile_pool(name="phase2", bufs=1))
        uvp = ctx2.enter_context(tc.tile_pool(name="uvp", bufs=1))

        # ---- flat uk load: partition (v,d) = contiguous 9216-elem span;
        # part boundaries align with window row-groups 0-63 / 64-127 so the
        # window copies below can start as soon as their parts land ----
        # only [0:6960] is ever read by the window views (max col 6927)
        uk_flat = P2.tile([12, CH], F32R, tag="uk")
        uk_cuts = (0, 2320, 4640, 6960)
        for i, eng in enumerate((nc.sync, nc.gpsimd, nc.scalar)):
            lo, hi = uk_cuts[i], uk_cuts[i + 1]
            src = AP(tensor=kp_all.tensor, offset=kp_all.offset + lo,
                     ap=[[3 * CH, 3], [DST, 4], [1, hi - lo]])
            eng.dma_start(out=uk_flat[:, lo:hi], in_=src.bitcast(F32R))

        # ---- flat uv load (SP/Pool only; keep ACT free for ukr0 + exp) ----
        uv_flat = P2.tile([12, CH], F32R, tag="uv")
        for i, eng in enumerate((nc.sync, nc.gpsimd)):
            lo, hi = 3480 * i, 3480 * (i + 1)
            src = AP(tensor=vp_all.tensor, offset=vp_all.offset + lo,
                     ap=[[3 * CH, 3], [DST, 4], [1, hi - lo]])
            eng.dma_start(out=uv_flat[:, lo:hi], in_=src.bitcast(F32R))

        # ---- materialize flat window views (stationary matmul operands
        # must be 1-free-dim): engine copies with strided window reads;
        # uvT [120, 13, 48]: slot 0 = ones (denominator), slots 1-12 =
        # PE-transposed [12,120] kv chunks ----
        ukb, uvb = uk_flat[:, 0:1], uv_flat[:, 0:1]
        id12r = id12[:, :]
        uk_rep, uvT = [], []
        with tc.tile_pool(name="pstv", bufs=4, space="PSUM") as pstv:
            for j in range(2):
                ukr = P.tile([12, KV], F32R, tag=f"ukr{j}")
                uk_src = AP(tensor=ukb.tensor, offset=ukb.offset + 24 * j,
                            ap=[ukb.ap[0], [48, M0], [1, M1]])
                uvr = uvp.tile([12, KV], F32R, tag="uvr")
                uv_src = AP(tensor=uvb.tensor, offset=uvb.offset + 24 * j,
                            ap=[uvb.ap[0], [48, M0], [1, M1]])
                if j == 0:
                    nc.scalar.copy(ukr[:, :], uk_src)
                    nc.vector.tensor_copy(uvr[:, :], uv_src)
                else:
                    nc.gpsimd.tensor_copy(uvr[:, :], uv_src)
                    nc.gpsimd.tensor_copy(ukr[:, :], uk_src)
                uk_rep.append(ukr)

                t = P.tile([KC, 13, NKC], F32R, tag=f"uvt{j}")
                nc.vector.memset(t[:, 0, :].bitcast(F32), 1.0)
                uvT.append(t)
                for c in range(NKC):
                    tp = pstv.tile([KC, 12], F32R, tag="tpv")
                    nc.tensor.transpose(
                        tp, uvr[:, KC * c : KC * (c + 1)], id12r)
                    nc.vector.tensor_copy(uvT[j][:, 1:13, c], tp)

        ctx2.close()  # free uk_flat / uv_flat / uvr SBUF
        PL = ctx.enter_context(tc.tile_pool(name="late", bufs=1))

        # ---- attention (qc-outer / j-inner) with the final conv
        # interleaved: after q-chunk t, output rows for conv chunks
        # c <= 2t are complete, so the 3x3 conv streams behind it ----
        out_sb = PL.tile([COUT, 128, 48], F32, tag="outsb")
        dma_engs = (nc.sync, nc.gpsimd, nc.scalar, nc.sync)
        rec_a = PL.tile([32, QC], F32, tag="reca")
        nc.vector.memset(rec_a, 0.0)
        rec_b = PL.tile([32, QC], F32, tag="recb")
        with (
            tc.tile_pool(name="psqk", bufs=2, space="PSUM") as psqk,
            tc.tile_pool(name="psav", bufs=1, space="PSUM") as psav,
            tc.tile_pool(name="psf", bufs=1, space="PSUM") as psf,
            tc.tile_pool(name="expp", bufs=2) as expp,
            tc.tile_pool(name="redp", bufs=2) as redp,
        ):
            ci = 0  # next final-conv chunk to emit (one-qc lag: deps stale)
            for qc in range(NQC):
                for j in range(2):
                    ps_av = psav.tile([13, QC], F32, tag="av")
                    for rnd in range(NKC // RPB):
                        ps_qk = psqk.tile([KC, RPB, 512], F32, tag="qk")
                        for b in range(RPB):
                            c = RPB * rnd + b
                            lhsT = uk_rep[j][:, KC * c : KC * (c + 1)]
                            rhs = q_sb[
                                :,
                                QROWS * qc : QROWS * (qc + 1),
                                24 * j : 24 * j + 24,
                            ]
                            out = ps_qk[0:KC, b, 0:QC].rearrange(
                                "p (a c) -> p a c", a=QROWS
                            )
                            nc.tensor.matmul(out, lhsT, rhs,
                                             start=True, stop=True)
                        ex = expp.tile([KC, RPB, QC], F32R, tag="ex")
                        nc.scalar.activation(
                            ex, ps_qk[0:KC, :, 0:QC],
                            mybir.ActivationFunctionType.Exp,
                        )
                        for b in range(RPB):
                            c = RPB * rnd + b
                            nc.tensor.matmul(
                                ps_av[:, :], uvT[j][:, :, c], ex[:, b, :],
                                start=(c == 0), stop=(c == NKC - 1),
                            )
                    # normalize: row 0 of ps_av is the softmax denominator;
                    # the per-q reciprocal broadcasts across partitions via
                    # a stride-0 AP on GPSIMD (software engine, SBUF only)
                    s0 = redp.tile([13, QC], F32, tag="s0")
                    nc.vector.tensor_copy(s0, ps_av[:, :])
                    nc.vector.reciprocal(rec_a[0:1, :], s0[0:1, :])
                    nc.vector.stream_shuffle(rec_b[:, :], rec_a[:, :],
                                             [0] * 32)
                    o_div = redp.tile([13, QC], F32, tag="odiv")
                    nc.vector.tensor_tensor(out=o_div, in0=s0[:, :],
                                            in1=rec_b[0:13, :],
                                            op=mybir.AluOpType.mult)
                    dst = o_pad[
                        :,
                        1 + QROWS * qc : 1 + QROWS * (qc + 1),
                        1 + 24 * j : 25 + 24 * j,
                    ]
                    nc.vector.tensor_copy(
                        dst, o_div.rearrange("p (a c) -> p a c", a=QROWS)
                    )
                    if j == 0:
                        # final-conv chunks for rows finished a full q-chunk
                        # ago: PE reaches them mid-qc with deps long stale
                        while ci <= 2 * (qc - 1):
                            ps = psf.tile([COUT, 8, 48], F32, tag="fps")
                            for t in range(9):
                                dy, dx = t // 3, t % 3
                                rhs = o_pad[:, 8 * ci + dy : 8 * ci + dy + 8,
                                            dx : dx + 48]
                                nc.tensor.matmul(
                                    ps[:, :, :], wo_sb[:, t, :], rhs,
                                    start=(t == 0), stop=(t == 8))
                            nc.vector.tensor_copy(
                                out_sb[:, 8 * ci : 8 * ci + 8, :], ps)
                            if ci % 4 == 3:
                                q4 = ci // 4
                                dma_engs[q4].dma_start(
                                    out=out_d[:, 1536 * q4 : 1536 * (q4 + 1)],
                                    in_=out_sb[:, 32 * q4 : 32 * (q4 + 1), :]
                                    .rearrange("p a b -> p (a b)"),
                                )
                            ci += 1
            # drain remaining final-conv chunks
            while ci <= 15:
                ps = psf.tile([COUT, 8, 48], F32, tag="fps")
                for t in range(9):
                    dy, dx = t // 3, t % 3
                    rhs = o_pad[:, 8 * ci + dy : 8 * ci + dy + 8,
                                dx : dx + 48]
                    nc.tensor.matmul(ps[:, :, :], wo_sb[:, t, :], rhs,
                                     start=(t == 0), stop=(t == 8))
                nc.vector.tensor_copy(out_sb[:, 8 * ci : 8 * ci + 8, :], ps)
                if ci % 4 == 3:  # stream the finished quarter out
                    q4 = ci // 4
                    dma_engs[q4].dma_start(
                        out=out_d[:, 1536 * q4 : 1536 * (q4 + 1)],
                        in_=out_sb[:, 32 * q4 : 32 * (q4 + 1), :]
                        .rearrange("p a b -> p (a b)"),
                    )
                ci += 1

    nc.compile()
    return nc


_NC = None


def _get_nc():
    global _NC
    if _NC is None:
        _NC = build_nc()
    return _NC


def make_in_maps(x, wq, bq, wk, bk, wv, bv, wo):
    x = np.asarray(x, np.float32)[0]           # [64, 128, 48]
    xp = np.zeros((CIN, 130, 50), np.float32)
    xp[:, 1:129, 1:49] = x
    xp = xp.reshape(CIN, -1)
    s = np.float32(DPH ** -0.5)

    def taps(w):                                # [O, I, 3, 3] -> [I, 9, O]
        return np.ascontiguousarray(np.transpose(w, (1, 2, 3, 0)).reshape(
            w.shape[1], 9, w.shape[0]))

    wq_np = np.asarray(wq, np.float32)
    wk_np = np.asarray(wk, np.float32) * s
    wv_np = np.asarray(wv, np.float32)
    wo_np = np.asarray(wo, np.float32)
    bq_np = np.asarray(bq, np.float32)
    bk_np = np.asarray(bk, np.float32) * s
    bv_np = np.asarray(bv, np.float32)

    in_maps = []
    for h in range(8):
        c_lo = (24576 * h) // 9216
        phi = (24576 * h - 9216 * c_lo) // 64
        v_idx = PHIS.index(phi)

        wqkv = np.zeros((48, CIN, 3, 3), np.float32)
        wqkv[4 * v_idx : 4 * v_idx + 4] = wq_np[4 * h : 4 * h + 4]
        wqkv[32:36] = wk_np[c_lo : c_lo + 4]
        wqkv[36 + 4 * v_idx : 36 + 4 * v_idx + 4] = wv_np[c_lo : c_lo + 4]

        b48 = np.zeros((48,), np.float32)
        b48[4 * v_idx : 4 * v_idx + 4] = bq_np[4 * h : 4 * h + 4]
        b48[32:36] = bk_np[c_lo : c_lo + 4]
        b48[36 + 4 * v_idx : 36 + 4 * v_idx + 4] = bv_np[c_lo : c_lo + 4]

        wo_t4 = np.ascontiguousarray(
            np.transpose(wo_np[:, 4 * h : 4 * h + 4], (1, 2, 3, 0))
        ).reshape(4, -1)
        wo13 = np.zeros((13, wo_t4.shape[1]), np.float32)
        wo13[1 + 4 * v_idx : 1 + 4 * v_idx + 4] = wo_t4

        in_maps.append({
            "xp": xp,
            "wqkv_t": taps(wqkv).reshape(CIN, -1),
            "b48": b48.reshape(48, 1),
            "wo_t": wo13,
            "id16": np.eye(16, dtype=np.float32),
            "id12": np.eye(12, dtype=np.float32),
            "ones13": np.ones((1, 13), np.float32),
        })
    return in_maps


def kernel(x, wq, bq, wk, bk, wv, bv, wo):
    from concourse.bass_utils import run_bass_kernel_spmd

    nc = _get_nc()
    in_maps = make_in_maps(x, wq, bq, wk, bk, wv, bv, wo)
    res = run_bass_kernel_spmd(nc, in_maps, list(range(8))).results
    out = np.zeros((COUT, H * W), np.float32)
    for m in res:
        out = out + m["out"]
    return out.reshape(1, COUT, H, W)


# revision 69
# speedup vs baseline: 2.0010x; 1.0059x over previous
"""Trainium2 Bass kernel for nn_MultiHeadAttention_75737453297867.

Sharding: one head per NeuronCore (8 heads / 8 cores). The reference's
aliased as_strided gather needs a per-core base offset 24576*h into the
flange-padded k/v storage; 24576*h mod 9216 is row-aligned (phi in
{0,48,96}), so three phi-shifted staging variants are built statically
and selection happens through host data alone: q and v convs run with
12 stacked channels (inactive variants host-zeroed), the QK contraction
runs over K=12, AV produces 13 rows (1 denominator + 12 variant-dph)
and the final conv weights (host-built) pick the active variant.

Staging path (cost model: DMA time = free bytes/partition * 0.386ns):
conv out [28,128,48] -> PE-transposed to row-major [128,16,48] ->
per-variant phi-shifted writes into flat DRAM ([144,64]-padded channel
images) -> one flat [12,9216] load back for uk (contiguous rows, no 2x
small-elem penalty) + direct strided DMAs building uvT [120,13,48].
QK reads uk chunks via strided APs (row-aliased windows), exp on ACT
from PSUM, AV accumulates [13,384] on PE, normalize writes straight
into the padded o image, final 3x3 conv, partial outputs summed on
host.
"""

import sys

import numpy as np

if "/opt/trn_rl_repo" not in sys.path:
    sys.path.insert(0, "/opt/trn_rl_repo")

import concourse.bass as bass
import concourse.tile as tile
from concourse import bacc
from concourse import mybir
from concourse.bass_types import AP

# Problem constants
CIN, COUT, H, W = 64, 64, 128, 48
DM, NH, DPH = 32, 8, 4
Q0, Q1, F0, F1 = 128, 24, 8, 8
M0, M1 = Q0 + 2 * F0, Q1 + 2 * F1          # 144, 40
KV = M0 * M1                               # 5760
CH = 144 * 64                              # 9216 flat padded-channel size
DST = 6144                                 # d-stride (Hp*Wp) in flat coords
KC = 128                                   # kv chunk: flat 128-slice of window
NKC = 45
RPB = 3                                    # kv chunks per psum round
NQC = 8                                    # q chunks of 384 (16 rows x 24)
QC = 384
QROWS = 16
PHIS = (0, 48, 96)
F32 = mybir.dt.float32
F32R = mybir.dt.float32r


def build_nc():
    nc = bacc.Bacc()

    xp_d = nc.dram_tensor("xp", [CIN, 130 * 50], F32R, kind="ExternalInput")
    wqkv_d = nc.dram_tensor("wqkv_t", [CIN, 9 * 48], F32R, kind="ExternalInput")
    b48_d = nc.dram_tensor("b48", [48, 1], F32, kind="ExternalInput")
    wo_d = nc.dram_tensor("wo_t", [13, 9 * 64], F32R, kind="ExternalInput")
    id16_d = nc.dram_tensor("id16", [16, 16], F32, kind="ExternalInput")
    id12_d = nc.dram_tensor("id12", [12, 12], F32R, kind="ExternalInput")
    o13_d = nc.dram_tensor("ones13", [1, 13], F32R, kind="ExternalInput")
    out_d = nc.dram_tensor("out", [COUT, H * W], F32, kind="ExternalOutput")

    from contextlib import ExitStack

    with tile.TileContext(nc) as tc, ExitStack() as ctx:
        P = ctx.enter_context(tc.tile_pool(name="persist", bufs=1))
        dram = ctx.enter_context(tc.tile_pool(name="dram", bufs=1, space="DRAM"))
        ctx1 = ctx.enter_context(ExitStack())
        P1 = ctx1.enter_context(tc.tile_pool(name="phase1", bufs=1))

        # ---- input loads (xp split across two DMA lanes) ----
        xp_sb = P1.tile([CIN, 130, 50], F32R, tag="xp")
        nc.sync.dma_start(
            out=xp_sb[:, 0:65, :],
            in_=xp_d[:, 0:3250].rearrange("p (a b) -> p a b", a=65),
        )
        nc.gpsimd.dma_start(
            out=xp_sb[:, 65:130, :],
            in_=xp_d[:, 3250:6500].rearrange("p (a b) -> p a b", a=65),
        )
        wqkv_sb = P.tile([CIN, 9, 48], F32R, tag="wqkv")
        nc.sync.dma_start(
            out=wqkv_sb, in_=wqkv_d[:, :].rearrange("p (t o) -> p t o", t=9)
        )
        b48 = P.tile([48, 1], F32, tag="b48")
        nc.sync.dma_start(out=b48, in_=b48_d[:, :])
        wo_sb = P.tile([13, 9, 64], F32R, tag="wo")
        nc.sync.dma_start(
            out=wo_sb, in_=wo_d[:, :].rearrange("p (t o) -> p t o", t=9)
        )
        id16 = P.tile([16, 16], F32, tag="id16")
        nc.sync.dma_start(out=id16, in_=id16_d[:, :])
        id12 = P.tile([12, 12], F32R, tag="id12")
        nc.sync.dma_start(out=id12, in_=id12_d[:, :])
        ones13 = P.tile([1, 13], F32R, tag="ones13")
        nc.sync.dma_start(out=ones13, in_=o13_d[:, :])

        zero_sb = P1.tile([128, 648], F32, tag="zeros")
        nc.vector.memset(zero_sb, 0.0)

        # ---- DRAM staging buffers (3 variants x 3 channels each) ----
        kp_all = dram.tile([9, CH], F32, tag="kp")
        vp_all = dram.tile([9, CH], F32, tag="vp")
        for buf in (kp_all, vp_all):
            dst = AP(tensor=buf.tensor, offset=buf.offset,
                     ap=[[648, 128], [1, 648]])
            nc.scalar.dma_start(out=dst, in_=zero_sb[:, :])

        # ---- stacked q/k/v conv: rows 0-11 = q, rows 32-47 = k4 + v12 ----
        # (kv starts at 32: engine PSUM access must be 32-partition aligned)
        q_sb = P.tile([12, 128, 48], F32R, tag="q_sb")
        kv_sb = P1.tile([16, 128, 48], F32, tag="kv_sb")
        with tc.tile_pool(name="psc", bufs=4, space="PSUM") as psc:
            for chv in range(16):
                ps = psc.tile([48, 8, 48], F32, tag="cps")
                for t in range(9):
                    dy, dx = t // 3, t % 3
                    rhs = xp_sb[:, 8 * chv + dy : 8 * chv + dy + 8, dx : dx + 48]
                    nc.tensor.matmul(
                        ps[:, :, :], wqkv_sb[:, t, 0:48], rhs,
                        start=(t == 0), stop=(t == 8),
                    )
                nc.vector.tensor_scalar_add(
                    q_sb[:, 8 * chv : 8 * chv + 8, :], ps[0:12, :, :],
                    b48[0:12, 0:1],
                )
                nc.scalar.add(
                    kv_sb[:, 8 * chv : 8 * chv + 8, :], ps[32:48, :, :],
                    b48[32:48, 0:1],
                )

        # ---- transpose k/v to row-major [128 rows, 16 ch, 48 cols] ----
        kv_row = P1.tile([128, 16, 48], F32, tag="kv_row")
        with tc.tile_pool(name="pst", bufs=2, space="PSUM") as pst:
            for x in range(48):
                tp = pst.tile([128, 16], F32, tag="tp")
                nc.tensor.matmul(tp, kv_sb[:, :, x], id16[:, :],
                                 start=True, stop=True)
                nc.vector.tensor_copy(kv_row[:, :, x], tp)

        # ---- phi-shifted staging writes into the padded channel images ----
        engs = [nc.sync, nc.gpsimd, nc.scalar]
        ei = 0
        for buf_all, cbase in ((kp_all, lambda v: 0), (vp_all, lambda v: 4 + 4 * v)):
            for v, phi in enumerate(PHIS):
                cb = cbase(v)
                base = buf_all.offset + 3 * v * CH
                if phi == 0:
                    dst = AP(tensor=buf_all.tensor, offset=base + 8 * 64 + 8,
                             ap=[[64, 128], [CH, 3], [1, 48]])
                    engs[ei % 3].dma_start(out=dst, in_=kv_row[0:128, cb : cb + 3, :])
                    ei += 1
                else:
                    n1 = 136 - phi
                    dst1 = AP(tensor=buf_all.tensor, offset=base + 8,
                              ap=[[64, n1], [CH, 3], [1, 48]])
                    engs[ei % 3].dma_start(
                        out=dst1, in_=kv_row[phi - 8 : 128, cb : cb + 3, :])
                    ei += 1
                    n2 = phi - 8
                    dst2 = AP(tensor=buf_all.tensor,
                              offset=base + (152 - phi) * 64 + 8,
                              ap=[[64, n2], [CH, 3], [1, 48]])
                    engs[ei % 3].dma_start(
                        out=dst2, in_=kv_row[0 : phi - 8, cb + 1 : cb + 4, :])
                    ei += 1

        # ---- padded attention-output image; zero only the 1-px border ----
        o_pad = P.tile([13, 130, 50], F32R, tag="opad")
        zb = zero_sb[0:13, 0:1]
        for dst in (o_pad[:, 0, :], o_pad[:, 129, :],
                    o_pad[:, 1:129, 0], o_pad[:, 1:129, 49]):
            n = dst.free_size()
            src = AP(tensor=zb.tensor, offset=zb.offset, ap=[zb.ap[0], [0, n]])
            nc.vector.tensor_copy(dst, src)

        ctx1.close()  # free xp / kv_sb / kv_row / zeros SBUF
        ctx2 = ctx.enter_context(ExitStack())
        P2 = ctx2.enter_context(tc.tile_pool(name="phase2", bufs=1))
        uvp = ctx2.enter_context(tc.tile_pool(name="uvp", bufs=1))

        # ---- flat uk load: partition (v,d) = contiguous 9216-elem span;
        # part boundaries align with window row-groups 0-63 / 64-127 so the
        # window copies below can start as soon as their parts land ----
        # only [0:6960] is ever read by the window views (max col 6927)
        uk_flat = P2.tile([12, CH], F32R, tag="uk")
        uk_cuts = (0, 2320, 4640, 6960)
        for i, eng in enumerate((nc.sync, nc.gpsimd, nc.scalar)):
            lo, hi = uk_cuts[i], uk_cuts[i + 1]
            src = AP(tensor=kp_all.tensor, offset=kp_all.offset + lo,
                     ap=[[3 * CH, 3], [DST, 4], [1, hi - lo]])
            eng.dma_start(out=uk_flat[:, lo:hi], in_=src.bitcast(F32R))

        # ---- flat uv load (SP/Pool only; keep ACT free for ukr0 + exp) ----
        uv_flat = P2.tile([12, CH], F32R, tag="uv")
        for i, eng in enumerate((nc.sync, nc.gpsimd)):
            lo, hi = 3480 * i, 3480 * (i + 1)
            src = AP(tensor=vp_all.tensor, offset=vp_all.offset + lo,
                     ap=[[3 * CH, 3], [DST, 4], [1, hi - lo]])
            eng.dma_start(out=uv_flat[:, lo:hi], in_=src.bitcast(F32R))

        # ---- materialize flat window views (stationary matmul operands
        # must be 1-free-dim): engine copies with strided window reads;
        # uvT [120, 13, 48]: slot 0 = ones (denominator), slots 1-12 =
        # PE-transposed [12,120] kv chunks ----
        ukb, uvb = uk_flat[:, 0:1], uv_flat[:, 0:1]
        id12r = id12[:, :]
        uk_rep, uvT = [], []
        with tc.tile_pool(name="pstv", bufs=4, space="PSUM") as pstv:
            for j in range(2):
                ukr = P.tile([12, KV], F32R, tag=f"ukr{j}")
                uk_src = AP(tensor=ukb.tensor, offset=ukb.offset + 24 * j,
                            ap=[ukb.ap[0], [48, M0], [1, M1]])
                uvr = uvp.tile([12, KV], F32R, tag="uvr")
                uv_src = AP(tensor=uvb.tensor, offset=uvb.offset + 24 * j,
                            ap=[uvb.ap[0], [48, M0], [1, M1]])
                # uv window copy split across DVE + Pool halves
                half = 2880  # 72 m0-rows
                uv_src_hi = AP(tensor=uvb.tensor,
                               offset=uvb.offset + 24 * j + 48 * 72,
                               ap=[uvb.ap[0], [48, M0 - 72], [1, M1]])
                nc.vector.tensor_copy(uvr[:, 0:half],
                                      AP(tensor=uvb.tensor,
                                         offset=uvb.offset + 24 * j,
                                         ap=[uvb.ap[0], [48, 72], [1, M1]]))
                nc.gpsimd.tensor_copy(uvr[:, half:KV], uv_src_hi)
                if j == 0:
                    nc.scalar.copy(ukr[:, :], uk_src)
                else:
                    nc.gpsimd.tensor_copy(ukr[:, :], uk_src)
                uk_rep.append(ukr)

                t = P.tile([KC, 13, NKC], F32R, tag=f"uvt{j}")
                nc.vector.memset(t[:, 0, :].bitcast(F32), 1.0)
                uvT.append(t)
                for c in range(NKC):
                    tp = pstv.tile([KC, 12], F32R, tag="tpv")
                    nc.tensor.transpose(
                        tp, uvr[:, KC * c : KC * (c + 1)], id12r)
                    nc.vector.tensor_copy(uvT[j][:, 1:13, c], tp)

        ctx2.close()  # free uk_flat / uv_flat / uvr SBUF
        PL = ctx.enter_context(tc.tile_pool(name="late", bufs=1))

        # ---- attention (qc-outer / j-inner) with the final conv
        # interleaved: after q-chunk t, output rows for conv chunks
        # c <= 2t are complete, so the 3x3 conv streams behind it ----
        out_sb = PL.tile([COUT, 128, 48], F32, tag="outsb")
        dma_engs = (nc.sync, nc.gpsimd, nc.scalar, nc.sync)
        rec_a = PL.tile([32, QC], F32, tag="reca")
        nc.vector.memset(rec_a, 0.0)
        rec_b = PL.tile([32, QC], F32, tag="recb")
        with (
            tc.tile_pool(name="psqk", bufs=2, space="PSUM") as psqk,
            tc.tile_pool(name="psav", bufs=1, space="PSUM") as psav,
            tc.tile_pool(name="psf", bufs=1, space="PSUM") as psf,
            tc.tile_pool(name="expp", bufs=6) as expp,
            tc.tile_pool(name="redp", bufs=2) as redp,
        ):
            ci = 0  # next final-conv chunk to emit (one-qc lag: deps stale)
            for qc in range(NQC):
                for j in range(2):
                    ps_av = psav.tile([13, QC], F32, tag="av")
                    for rnd in range(NKC // RPB):
                        ps_qk = psqk.tile([KC, RPB, 512], F32, tag="qk")
                        for b in range(RPB):
                            c = RPB * rnd + b
                            lhsT = uk_rep[j][:, KC * c : KC * (c + 1)]
                            rhs = q_sb[
                                :,
                                QROWS * qc : QROWS * (qc + 1),
                                24 * j : 24 * j + 24,
                            ]
                            out = ps_qk[0:KC, b, 0:QC].rearrange(
                                "p (a c) -> p a c", a=QROWS
                            )
                            nc.tensor.matmul(out, lhsT, rhs,
                                             start=True, stop=True)
                        ex = expp.tile([KC, RPB, QC], F32R, tag="ex")
                        nc.scalar.activation(
                            ex, ps_qk[0:KC, :, 0:QC],
                            mybir.ActivationFunctionType.Exp,
                        )
                        for b in range(RPB):
                            c = RPB * rnd + b
                            nc.tensor.matmul(
                                ps_av[:, :], uvT[j][:, :, c], ex[:, b, :],
                                start=(c == 0), stop=(c == NKC - 1),
                            )
                    # normalize: row 0 of ps_av is the softmax denominator;
                    # the per-q reciprocal broadcasts across partitions via
                    # a stride-0 AP on GPSIMD (software engine, SBUF only)
                    s0 = redp.tile([13, QC], F32, tag="s0")
                    nc.vector.tensor_copy(s0, ps_av[:, :])
                    nc.vector.reciprocal(rec_a[0:1, :], s0[0:1, :])
                    nc.vector.stream_shuffle(rec_b[:, :], rec_a[:, :],
                                             [0] * 32)
                    o_div = redp.tile([13, QC], F32, tag="odiv")
                    nc.vector.tensor_tensor(out=o_div, in0=s0[:, :],
                                            in1=rec_b[0:13, :],
                                            op=mybir.AluOpType.mult)
                    dst = o_pad[
                        :,
                        1 + QROWS * qc : 1 + QROWS * (qc + 1),
                        1 + 24 * j : 25 + 24 * j,
                    ]
                    nc.vector.tensor_copy(
                        dst, o_div.rearrange("p (a c) -> p a c", a=QROWS)
                    )
                    if j == 0:
                        # final-conv chunks for rows finished a full q-chunk
                        # ago: PE reaches them mid-qc with deps long stale
                        while ci <= 2 * (qc - 1):
                            ps = psf.tile([COUT, 8, 48], F32, tag="fps")
                            for t in range(9):
                                dy, dx = t // 3, t % 3
                                rhs = o_pad[:, 8 * ci + dy : 8 * ci + dy + 8,
                                            dx : dx + 48]
                                nc.tensor.matmul(
                                    ps[:, :, :], wo_sb[:, t, :], rhs,
                                    start=(t == 0), stop=(t == 8))
                            nc.vector.tensor_copy(
                                out_sb[:, 8 * ci : 8 * ci + 8, :], ps)
                            if ci % 4 == 3:
                                q4 = ci // 4
                                dma_engs[q4].dma_start(
                                    out=out_d[:, 1536 * q4 : 1536 * (q4 + 1)],
                                    in_=out_sb[:, 32 * q4 : 32 * (q4 + 1), :]
                                    .rearrange("p a b -> p (a b)"),
                                )
                            ci += 1
            # drain remaining final-conv chunks
            while ci <= 15:
                ps = psf.tile([COUT, 8, 48], F32, tag="fps")
                for t in range(9):
                    dy, dx = t // 3, t % 3
                    rhs = o_pad[:, 8 * ci + dy : 8 * ci + dy + 8,
                                dx : dx + 48]
                    nc.tensor.matmul(ps[:, :, :], wo_sb[:, t, :], rhs,
                                     start=(t == 0), stop=(t == 8))
                nc.vector.tensor_copy(out_sb[:, 8 * ci : 8 * ci + 8, :], ps)
                if ci % 4 == 3:  # stream the finished quarter out
                    q4 = ci // 4
                    dma_engs[q4].dma_start(
                        out=out_d[:, 1536 * q4 : 1536 * (q4 + 1)],
                        in_=out_sb[:, 32 * q4 : 32 * (q4 + 1), :]
                        .rearrange("p a b -> p (a b)"),
                    )
                ci += 1

    nc.compile()
    return nc


_NC = None


def _get_nc():
    global _NC
    if _NC is None:
        _NC = build_nc()
    return _NC


def make_in_maps(x, wq, bq, wk, bk, wv, bv, wo):
    x = np.asarray(x, np.float32)[0]           # [64, 128, 48]
    xp = np.zeros((CIN, 130, 50), np.float32)
    xp[:, 1:129, 1:49] = x
    xp = xp.reshape(CIN, -1)
    s = np.float32(DPH ** -0.5)

    def taps(w):                                # [O, I, 3, 3] -> [I, 9, O]
        return np.ascontiguousarray(np.transpose(w, (1, 2, 3, 0)).reshape(
            w.shape[1], 9, w.shape[0]))

    wq_np = np.asarray(wq, np.float32)
    wk_np = np.asarray(wk, np.float32) * s
    wv_np = np.asarray(wv, np.float32)
    wo_np = np.asarray(wo, np.float32)
    bq_np = np.asarray(bq, np.float32)
    bk_np = np.asarray(bk, np.float32) * s
    bv_np = np.asarray(bv, np.float32)

    in_maps = []
    for h in range(8):
        c_lo = (24576 * h) // 9216
        phi = (24576 * h - 9216 * c_lo) // 64
        v_idx = PHIS.index(phi)

        wqkv = np.zeros((48, CIN, 3, 3), np.float32)
        wqkv[4 * v_idx : 4 * v_idx + 4] = wq_np[4 * h : 4 * h + 4]
        wqkv[32:36] = wk_np[c_lo : c_lo + 4]
        wqkv[36 + 4 * v_idx : 36 + 4 * v_idx + 4] = wv_np[c_lo : c_lo + 4]

        b48 = np.zeros((48,), np.float32)
        b48[4 * v_idx : 4 * v_idx + 4] = bq_np[4 * h : 4 * h + 4]
        b48[32:36] = bk_np[c_lo : c_lo + 4]
        b48[36 + 4 * v_idx : 36 + 4 * v_idx + 4] = bv_np[c_lo : c_lo + 4]

        wo_t4 = np.ascontiguousarray(
            np.transpose(wo_np[:, 4 * h : 4 * h + 4], (1, 2, 3, 0))
        ).reshape(4, -1)
        wo13 = np.zeros((13, wo_t4.shape[1]), np.float32)
        wo13[1 + 4 * v_idx : 1 + 4 * v_idx + 4] = wo_t4

        in_maps.append({
            "xp": xp,
            "wqkv_t": taps(wqkv).reshape(CIN, -1),
            "b48": b48.reshape(48, 1),
            "wo_t": wo13,
            "id16": np.eye(16, dtype=np.float32),
            "id12": np.eye(12, dtype=np.float32),
            "ones13": np.ones((1, 13), np.float32),
        })
    return in_maps


def kernel(x, wq, bq, wk, bk, wv, bv, wo):
    from concourse.bass_utils import run_bass_kernel_spmd

    nc = _get_nc()
    in_maps = make_in_maps(x, wq, bq, wk, bk, wv, bv, wo)
    res = run_bass_kernel_spmd(nc, in_maps, list(range(8))).results
    out = np.zeros((COUT, H * W), np.float32)
    for m in res:
        out = out + m["out"]
    return out.reshape(1, COUT, H, W)
